# revision 1
# baseline (speedup 1.0000x reference)
"""BitNet attention block on 8 TRN2 NeuronCores (tensor-parallel over heads).

Self-contained: kernel(**inputs) takes full inputs, shards internally,
runs one SPMD Bass program on cores 0-7, reassembles the full output.

Sharding: core c owns Q heads [4c,4c+4), KV head c, o_proj output dims
[512c, 512c+512). Attention is fully local per core (GQA groups align
with the sharding). Cross-core comms: two tiny AllReduces for the BitNet
absmean gammas and two AllGathers (one per batch) of the attention
output so each core can compute its o_proj output-dim shard for all
tokens. Host does layout transforms (transpose/shard/concat) only.

All matmuls run as float32r (full-rate fp32 path on the PE, needs
moving dim >= 512). Weights are DMA'd once and quantized in place; the
round-clip chain matches jnp.round half-to-even exactly via the
+1.5*2^23 magic-constant trick and is pipelined across the Scalar,
Vector and GpSimd engines. All gamma scale factors are folded into the
quantized weight values.
"""
import os
import sys
sys.path.insert(0, "/opt/trn_rl_repo")
import numpy as np

B, S, H = 2, 1024, 4096
NH, NKV, HD = 32, 8, 128
NCORES = 8
T = B * S
QH = NH // NCORES          # 4 q-heads per core
MSH = H // NCORES          # 512 o_proj out-dims per core
THETA = 10000.0
C_MAGIC = 12582912.0       # 1.5 * 2**23: (x + C) - C == round-half-even(x)
TWO_PI = 6.283185307179586
NKT = H // 128             # 32 contraction tiles
NTC = T // 512             # 4 token chunks
SKT = S // 128             # 8 score k-tiles per batch
SQC = S // 512             # 2 q-chunks per batch

_cache = {}
last_exec_time_ns = None


def _classify_mask(mask):
    """Per (b, kt, qc) [128k x 512q] block: 0 zero, 1 masked-out, 2 general.

    A block whose mask values are all <= -1e4 contributes exactly 0 to the
    fp32 softmax (exp underflows to 0.0), so it is skipped entirely.
    """
    status = np.empty((B, SKT, SQC), dtype=np.int8)
    index = {}
    packed = []
    for b in range(B):
        mb = np.asarray(mask[b, 0], dtype=np.float32)   # (q, k)
        for kt in range(SKT):
            for qc in range(SQC):
                blk = mb[qc * 512:(qc + 1) * 512, kt * 128:(kt + 1) * 128]
                if not blk.any():
                    status[b, kt, qc] = 0
                elif (blk <= -1e4).all():
                    status[b, kt, qc] = 1
                else:
                    status[b, kt, qc] = 2
                    index[(b, kt, qc)] = len(packed)
                    packed.append(np.ascontiguousarray(blk.T))  # (128k, 512q)
    if packed:
        packed_arr = np.concatenate(packed, axis=0).astype(np.float32)
    else:
        packed_arr = np.zeros((128, 512), dtype=np.float32)
    return status, index, packed_arr


def _cody_consts():
    c1 = float(np.float32(6.28125))
    r = np.float64(TWO_PI) - c1
    c2 = float(np.float32(r - np.remainder(r, 2.0 ** -24)))
    c3 = float(np.float32(np.float64(TWO_PI) - c1 - float(c2)))
    return c1, c2, c3


def _build(status, index, n_packed):
    from concourse import bacc, tile, mybir

    F32 = mybir.dt.float32
    F32R = mybir.dt.float32r
    ACTF = mybir.ActivationFunctionType
    ALU = mybir.AluOpType
    X = mybir.AxisListType.X
    RG = [list(range(NCORES))]
    c1, c2, c3 = _cody_consts()

    nc = bacc.Bacc("TRN2", target_bir_lowering=False, debug=False,
                   num_devices=NCORES)

    xT = nc.dram_tensor("xT", [H, T], F32, kind="ExternalInput")
    wqT = nc.dram_tensor("wqT", [H, QH * HD], F32, kind="ExternalInput")
    wkT = nc.dram_tensor("wkT", [H, HD], F32, kind="ExternalInput")
    wvT = nc.dram_tensor("wvT", [H, HD], F32, kind="ExternalInput")
    woT = nc.dram_tensor("woT", [H, MSH], F32, kind="ExternalInput")
    maskP = nc.dram_tensor("maskP", [n_packed * 128, 512], F32,
                           kind="ExternalInput")
    pos = nc.dram_tensor("pos", [1, T], F32, kind="ExternalInput")
    outN = nc.dram_tensor("outN", [T, MSH], F32, kind="ExternalOutput")

    idn_c = nc.inline_tensor(np.eye(128, dtype=np.float32), name="idn_c")
    onesk_c = nc.inline_tensor(np.ones((128, 1), np.float32), name="onesk_c")
    onesm_c = nc.inline_tensor(np.ones((1, 128), np.float32), name="onesm_c")
    invf_np = (1.0 / THETA ** (np.arange(0, HD, 2, dtype=np.float32) / HD))
    invf_np = np.concatenate([invf_np, invf_np]).reshape(HD, 1)
    invf_c = nc.inline_tensor(invf_np.astype(np.float32), name="invf_c")

    NQ = float(NH * HD * H)
    NK = float(NKV * HD * H)
    NO = float(H * NH * HD)

    with tile.TileContext(nc) as tc, \
         nc.allow_low_precision(reason="float32r kernel"):
        with tc.tile_pool(name="cpool", bufs=1) as cpool, \
             tc.tile_pool(name="dbounce", bufs=1, space="DRAM") as dbounce:
            # DRAM bounce tiles (tracked by Tile for collective deps)
            arq_in = dbounce.tile([1, 8], F32, name="arq_in")
            arq_out = dbounce.tile([1, 8], F32, name="arq_out",
                                   addr_space="Shared")
            ark_in = dbounce.tile([1, 8], F32, name="ark_in")
            ark_out = dbounce.tile([1, 8], F32, name="ark_out",
                                   addr_space="Shared")
            agin = [dbounce.tile([QH * HD, S], F32, name=f"agin{b}")
                    for b in range(B)]
            agout = [dbounce.tile([H, S], F32, name=f"agout{b}",
                                  addr_space="Shared") for b in range(B)]
            woq_dram = dbounce.tile([H, MSH], F32R, name="woq_dram")

            idn = cpool.tile([128, 128], F32R, name="idn")
            nc.sync.dma_start(out=idn[:], in_=idn_c[:, :].bitcast(F32R))
            oneskr = cpool.tile([128, 1], F32R, name="oneskr")
            nc.sync.dma_start(out=oneskr[:], in_=onesk_c[:, :].bitcast(F32R))
            onesk = cpool.tile([128, 1], F32, name="onesk")
            nc.sync.dma_start(out=onesk[:], in_=onesk_c[:, :])
            onesmr = cpool.tile([1, 128], F32R, name="onesmr")
            nc.sync.dma_start(out=onesmr[:], in_=onesm_c[:, :].bitcast(F32R))
            onesm = cpool.tile([1, 128], F32, name="onesm")
            nc.sync.dma_start(out=onesm[:], in_=onesm_c[:, :])
            invf = cpool.tile([128, 1], F32, name="invf")
            nc.sync.dma_start(out=invf[:], in_=invf_c[:, :])
            cmag = cpool.tile([128, 1], F32, name="cmag")
            nc.vector.memset(cmag[:], C_MAGIC)
            ones_full = cpool.tile([128, 512], F32, name="ones_full")
            nc.vector.memset(ones_full[:], 1.0)
            bsc = cpool.tile([128, 8], F32, name="bsc")

            with tc.tile_pool(name="qkv", bufs=1) as qkvp:
                wbig = tc.alloc_tile_pool(name="wbig", bufs=1)
                wq_sb = wbig.tile([128, NKT * 512], F32R, name="wq_sb")
                wbig2 = tc.alloc_tile_pool(name="wbig2", bufs=1)
                wk_sb = wbig2.tile([128, NKT * 128], F32R, name="wk_sb")
                wv_sb = wbig2.tile([128, NKT * 128], F32R, name="wv_sb")

                with tc.tile_pool(name="gacc", bufs=1) as gacc, \
                     tc.tile_pool(name="qscr", bufs=5) as qscr, \
                     tc.tile_pool(name="wopre", bufs=4) as wopre, \
                     tc.tile_pool(name="woqp", bufs=3) as woqp, \
                     tc.tile_pool(name="pgam", bufs=2, space="PSUM") as pgam:
                    # ---- single-pass: DMA W into final storage + |.| sums
                    # (wo is streamed twice instead: no room to stage it)
                    g4 = gacc.tile([128, 4], F32, name="g4")
                    specs = [(wqT, 512, wq_sb, True),
                             (wkT, 128, wk_sb, True),
                             (wvT, 128, wv_sb, True),
                             (woT, 512, None, False)]

                    def w_slice(wi, i, f32view):
                        wt, fw, dst, isr = specs[wi]
                        sl = dst[:, i * fw:(i + 1) * fw]
                        if isr and f32view:
                            return sl.bitcast(F32)
                        return sl

                    def prepass(wi):
                        wt, fw, dst, isr = specs[wi]
                        acc = gacc.tile([128, NKT], F32, name=f"acc{wi}")
                        eng = nc.gpsimd if wi == 3 else nc.sync
                        for i in range(NKT):
                            slf = wopre.tile([128, fw], F32,
                                             name=f"wp_{wi}_{i}",
                                             tag=f"wop{fw}")[:]
                            eng.dma_start(
                                out=slf, in_=wt[i * 128:(i + 1) * 128, :])
                            nc.vector.tensor_reduce(
                                acc[:, i:i + 1], slf, X, ALU.add,
                                apply_absolute_value=True)
                        nc.vector.tensor_reduce(g4[:, wi:wi + 1], acc[:], X,
                                                ALU.add)

                    # q/k/v prepass first: their AllReduce + quant gate
                    # phase 1. Wo's gamma goes in a second AllReduce that
                    # overlaps phase 1 (wo isn't needed until phase 3).
                    prepass(0)
                    prepass(1)
                    prepass(2)
                    pg_q = pgam.tile([1, 3], F32, name="pg_q")
                    nc.tensor.matmul(pg_q[:], onesk[:], g4[:, 0:3],
                                     start=True, stop=True)
                    gq_sb = gacc.tile([1, 8], F32, name="gq_sb")
                    nc.vector.memset(gq_sb[:], 0.0)
                    nc.scalar.copy(gq_sb[:, 0:3], pg_q[:])
                    nc.sync.dma_start(out=arq_in[:], in_=gq_sb[:])
                    nc.gpsimd.collective_compute(
                        "AllReduce", ALU.add, replica_groups=RG,
                        ins=[arq_in[:].opt()], outs=[arq_out[:].opt()])
                    arq_sb = gacc.tile([1, 8], F32, name="arq_sb")
                    nc.sync.dma_start(out=arq_sb[:], in_=arq_out[:])

                    prepass(3)
                    pg_k = pgam.tile([1, 1], F32, name="pg_k")
                    nc.tensor.matmul(pg_k[:], onesk[:], g4[:, 3:4],
                                     start=True, stop=True)
                    gk_sb = gacc.tile([1, 8], F32, name="gk_sb")
                    nc.vector.memset(gk_sb[:], 0.0)
                    nc.scalar.copy(gk_sb[:, 0:1], pg_k[:])
                    nc.sync.dma_start(out=ark_in[:], in_=gk_sb[:])
                    nc.gpsimd.collective_compute(
                        "AllReduce", ALU.add, replica_groups=RG,
                        ins=[ark_in[:].opt()], outs=[ark_out[:].opt()])
                    ark_sb = gacc.tile([1, 8], F32, name="ark_sb")
                    nc.sync.dma_start(out=ark_sb[:], in_=ark_out[:])

                    # gammas / broadcast scalars. q/k/v side from AR1
                    # (1/sqrt(HD) folded into the K scale); o side from AR2.
                    gamq = gacc.tile([1, 6], F32, name="gamq")
                    nc.vector.tensor_scalar(gamq[:, 3:4], arq_sb[:, 0:1],
                                            1.0 / NQ, 1e-5, ALU.mult,
                                            ALU.add)
                    nc.vector.tensor_scalar(gamq[:, 4:6], arq_sb[:, 1:3],
                                            1.0 / NK, 1e-5, ALU.mult,
                                            ALU.add)
                    nc.vector.reciprocal(gamq[:, 0:3], gamq[:, 3:6])
                    nc.vector.tensor_scalar(gamq[:, 4:5], gamq[:, 4:5],
                                            float(1.0 / np.sqrt(HD)), None,
                                            ALU.mult)
                    pgb1 = pgam.tile([128, 6], F32, name="pgb1")
                    nc.tensor.matmul(pgb1[:], onesm[:], gamq[:], start=True,
                                     stop=True)
                    nc.scalar.copy(bsc[:, 0:6], pgb1[:])
                    BIQ, BIK, BIV = (bsc[:, i:i + 1] for i in range(3))
                    BGQ, BSK, BGV = (bsc[:, i:i + 1] for i in range(3, 6))

                    gamo = gacc.tile([1, 2], F32, name="gamo")
                    nc.vector.tensor_scalar(gamo[:, 1:2], ark_sb[:, 0:1],
                                            1.0 / NO, 1e-5, ALU.mult,
                                            ALU.add)
                    nc.vector.reciprocal(gamo[:, 0:1], gamo[:, 1:2])
                    pgb2 = pgam.tile([128, 2], F32, name="pgb2")
                    nc.tensor.matmul(pgb2[:], onesm[:], gamo[:], start=True,
                                     stop=True)
                    nc.scalar.copy(bsc[:, 6:8], pgb2[:])
                    BIO, BGO = bsc[:, 6:7], bsc[:, 7:8]

                    # ---- in-place ternary quant, pipelined ACT->DVE->POOL
                    def quant_ip(wi, inv_ap, scale_ap):
                        wt, fw, dst, isr = specs[wi]
                        for i in range(NKT):
                            src = wopre.tile([128, fw], F32,
                                             name=f"w2_{wi}_{i}",
                                             tag=f"wop{fw}")[:]
                            nc.scalar.dma_start(
                                out=src,
                                in_=wt[i * 128:(i + 1) * 128, :])
                            s = qscr.tile([128, fw], F32,
                                          name=f"qs_{wi}_{i}", tag=f"qs{fw}")
                            # t = w*(1/gamma) + C  (sets up the exact round)
                            nc.scalar.activation(s[:], src, ACTF.Identity,
                                                 bias=cmag[:], scale=inv_ap)
                            # t = min(t - C, 1)
                            nc.vector.scalar_tensor_tensor(
                                s[:], s[:], C_MAGIC, ones_full[:, 0:fw],
                                ALU.subtract, ALU.min)
                            # w_q = max(t, -1) * scale
                            if wi == 3:
                                dstr = woqp.tile([128, fw], F32R,
                                                 name=f"woq_{i}",
                                                 tag="woqt")[:]
                            else:
                                dstr = w_slice(wi, i, False)
                            nc.vector.tensor_scalar(dstr, s[:], -1.0,
                                                    scale_ap, ALU.max,
                                                    ALU.mult)
                            if wi == 3:
                                nc.sync.dma_start(
                                    out=woq_dram[i * 128:(i + 1) * 128, :],
                                    in_=dstr)

                    quant_ip(0, BIQ, BGQ)
                    quant_ip(1, BIK, BSK)
                    quant_ip(2, BIV, BGV)
                    quant_ip(3, BIO, BGO)

                if True:
                    qT_sb = [qkvp.tile([128, T], F32R, name=f"qT{h}")
                             for h in range(QH)]
                    kT_sb = qkvp.tile([128, T], F32R, name="kT_sb")
                    vT_sb = qkvp.tile([128, T], F32R, name="vT_sb")

                    with tc.tile_pool(name="tab", bufs=1) as tab:
                        cos_sb = tab.tile([128, T], F32, name="cos_sb")
                        ss_sb = tab.tile([128, T], F32, name="ss_sb")
                        # RoPE tables: Cody-Waite range reduction + Sin
                        with tc.tile_pool(name="rtab", bufs=3) as rtab, \
                             tc.tile_pool(name="prt", bufs=2,
                                          space="PSUM") as prt:
                            for tcn in range(NTC):
                                cs = slice(tcn * 512, (tcn + 1) * 512)
                                pchunk = rtab.tile([1, 512], F32,
                                                   name=f"pos{tcn}",
                                                   tag="pos")
                                nc.sync.dma_start(out=pchunk[:],
                                                  in_=pos[0:1, cs])
                                pf = prt.tile([128, 512], F32,
                                              name=f"pf{tcn}", tag="pf")
                                nc.tensor.matmul(pf[:], onesm[:], pchunk[:],
                                                 start=True, stop=True)
                                f_sb = rtab.tile([128, 512], F32,
                                                 name=f"f{tcn}", tag="f")
                                nc.scalar.activation(f_sb[:], pf[:],
                                                     ACTF.Copy,
                                                     scale=invf[:])
                                k_sb = rtab.tile([128, 512], F32,
                                                 name=f"kk{tcn}", tag="kk")
                                nc.vector.tensor_scalar(k_sb[:], f_sb[:],
                                                        1.0 / TWO_PI,
                                                        C_MAGIC, ALU.mult,
                                                        ALU.add)
                                nc.vector.tensor_scalar(k_sb[:], k_sb[:],
                                                        C_MAGIC, None,
                                                        ALU.subtract)
                                y_sb = rtab.tile([128, 512], F32,
                                                 name=f"y{tcn}", tag="y")
                                nc.vector.scalar_tensor_tensor(
                                    y_sb[:], k_sb[:], -c1, f_sb[:],
                                    ALU.mult, ALU.add)
                                nc.vector.scalar_tensor_tensor(
                                    y_sb[:], k_sb[:], -c2, y_sb[:],
                                    ALU.mult, ALU.add)
                                nc.vector.scalar_tensor_tensor(
                                    y_sb[:], k_sb[:], -c3, y_sb[:],
                                    ALU.mult, ALU.add)
                                nc.scalar.activation(ss_sb[0:64, cs],
                                                     y_sb[0:64, :],
                                                     ACTF.Sin, scale=-1.0)
                                nc.scalar.activation(ss_sb[64:128, cs],
                                                     y_sb[64:128, :],
                                                     ACTF.Sin)
                                yc = rtab.tile([128, 512], F32,
                                               name=f"yc{tcn}", tag="yc")
                                nc.vector.tensor_scalar(yc[:], y_sb[:],
                                                        float(np.pi / 2),
                                                        None, ALU.add)
                                m_sb = rtab.tile([128, 512], F32,
                                                 name=f"mm{tcn}", tag="mm")
                                nc.vector.tensor_scalar(m_sb[:], yc[:],
                                                        float(np.pi), None,
                                                        ALU.is_gt)
                                nc.vector.scalar_tensor_tensor(
                                    yc[:], m_sb[:], -TWO_PI, yc[:],
                                    ALU.mult, ALU.add)
                                nc.scalar.activation(cos_sb[:, cs], yc[:],
                                                     ACTF.Sin)

                        # ---- phase 1: QKV projections + RoPE ----
                        with tc.tile_pool(name="xin", bufs=4) as xin, \
                             tc.tile_pool(name="rope", bufs=2) as rope, \
                             tc.tile_pool(name="p1", bufs=8,
                                          space="PSUM") as p1:
                            for tcn in range(NTC):
                                cs = slice(tcn * 512, (tcn + 1) * 512)
                                pq = [p1.tile([128, 512], F32,
                                              name=f"pq{tcn}_{h}", tag="p1")
                                      for h in range(QH)]
                                pk = p1.tile([128, 512], F32,
                                             name=f"pk{tcn}", tag="p1")
                                pv = p1.tile([128, 512], F32,
                                             name=f"pv{tcn}", tag="p1")
                                for kt in range(NKT):
                                    xt = xin.tile([128, 512], F32R,
                                                  name=f"x{tcn}_{kt}",
                                                  tag="xt")
                                    nc.sync.dma_start(
                                        out=xt[:],
                                        in_=xT[kt * 128:(kt + 1) * 128, cs]
                                        .bitcast(F32R))
                                    st, sp = (kt == 0), (kt == NKT - 1)
                                    for h in range(QH):
                                        nc.tensor.matmul(
                                            pq[h][:],
                                            wq_sb[:, kt * 512 + h * 128:
                                                  kt * 512 + (h + 1) * 128],
                                            xt[:], start=st, stop=sp,
                                            skip_group_check=True)
                                    nc.tensor.matmul(
                                        pk[:],
                                        wk_sb[:, kt * 128:(kt + 1) * 128],
                                        xt[:], start=st, stop=sp,
                                        skip_group_check=True)
                                    nc.tensor.matmul(
                                        pv[:],
                                        wv_sb[:, kt * 128:(kt + 1) * 128],
                                        xt[:], start=st, stop=sp,
                                        skip_group_check=True)

                                def rope_apply(psrc, dst_ap, tg):
                                    m1 = rope.tile([128, 512], F32,
                                                   name=f"m1{tg}", tag="m1")
                                    nc.vector.tensor_mul(m1[:], psrc[:],
                                                         cos_sb[:, cs])
                                    m2 = rope.tile([128, 512], F32,
                                                   name=f"m2{tg}", tag="m2")
                                    nc.vector.tensor_mul(m2[0:64, :],
                                                         psrc[64:128, :],
                                                         ss_sb[0:64, cs])
                                    nc.vector.tensor_mul(m2[64:128, :],
                                                         psrc[0:64, :],
                                                         ss_sb[64:128, cs])
                                    nc.vector.tensor_add(dst_ap, m1[:],
                                                         m2[:])
                                for h in range(QH):
                                    rope_apply(pq[h], qT_sb[h][:, cs],
                                               f"_{tcn}_{h}")
                                rope_apply(pk, kT_sb[:, cs], f"k_{tcn}")
                                nc.scalar.copy(vT_sb[:, cs], pv[:])

                    wbig2.release()
                    wbig.release()  # free wq/wk/wv SBUF before attention
                    # prefetch quantized wo back into SBUF during attention
                    w3 = tc.alloc_tile_pool(name="w3", bufs=1)
                    wo_sb = w3.tile([128, NKT * MSH], F32R, name="wo_sb")
                    for i in range(NKT):
                        nc.sync.dma_start(
                            out=wo_sb[:, i * MSH:(i + 1) * MSH],
                            in_=woq_dram[i * 128:(i + 1) * 128, :])
                    # ---- phase 2: attention ----
                    with tc.tile_pool(name="vnatp", bufs=2) as vnatp, \
                         tc.tile_pool(name="epool", bufs=8) as epool, \
                         tc.tile_pool(name="mpool", bufs=1) as mpool, \
                         tc.tile_pool(name="aop", bufs=4) as aop, \
                         tc.tile_pool(name="zpool", bufs=2) as zpool, \
                         tc.tile_pool(name="ps_s", bufs=2,
                                      space="PSUM") as ps_s, \
                         tc.tile_pool(name="ps_o", bufs=2,
                                      space="PSUM") as ps_o, \
                         tc.tile_pool(name="ps_x", bufs=1,
                                      space="PSUM") as ps_x:
                        for b in range(B):
                            boff = b * S
                            vnat = vnatp.tile([128, S], F32R,
                                              name=f"vnat{b}", tag="vnat")
                            for kt in range(SKT):
                                ptr = ps_x.tile([128, 128], F32R,
                                                name=f"ptr{b}_{kt}",
                                                tag="ptr")
                                nc.tensor.transpose(
                                    ptr[:],
                                    vT_sb[:, boff + kt * 128:
                                          boff + (kt + 1) * 128], idn[:])
                                nc.scalar.copy(
                                    vnat[:, kt * 128:(kt + 1) * 128],
                                    ptr[:].bitcast(F32))
                            for qc in range(SQC):
                                # mask tiles shared across the 4 heads
                                mtiles = {}
                                for kt in range(SKT):
                                    if status[b, kt, qc] == 2:
                                        mi = index[(b, kt, qc)]
                                        mt_ = mpool.tile(
                                            [128, 512], F32,
                                            name=f"mt{b}{qc}{kt}",
                                            tag=f"mt{kt}")
                                        nc.gpsimd.dma_start(
                                            out=mt_[:],
                                            in_=maskP[mi * 128:
                                                      (mi + 1) * 128, :])
                                        mtiles[kt] = mt_
                                kts = [kt for kt in range(SKT)
                                       if status[b, kt, qc] != 1]
                                assert kts, "fully-masked softmax row"
                                for h in range(QH):
                                    qsl = qT_sb[h][:, boff + qc * 512:
                                                   boff + (qc + 1) * 512]
                                    pz = ps_x.tile([1, 512], F32,
                                                   name=f"pz{b}{h}{qc}",
                                                   tag="pz")
                                    po = ps_o.tile([128, 512], F32,
                                                   name=f"po{b}{h}{qc}",
                                                   tag="po")
                                    for i, kt in enumerate(kts):
                                        ps_ = ps_s.tile(
                                            [128, 512], F32,
                                            name=f"s{b}{h}{qc}{kt}",
                                            tag="ps")
                                        nc.tensor.matmul(
                                            ps_[:],
                                            kT_sb[:, boff + kt * 128:
                                                  boff + (kt + 1) * 128],
                                            qsl, start=True, stop=True,
                                            skip_group_check=True)
                                        if kt in mtiles:
                                            nc.vector.tensor_add(
                                                ps_[:], ps_[:],
                                                mtiles[kt][:])
                                        e = epool.tile(
                                            [128, 512], F32R,
                                            name=f"e{b}{h}{qc}{kt}",
                                            tag="e")
                                        nc.scalar.activation(e[:], ps_[:],
                                                             ACTF.Exp)
                                        fst = (i == 0)
                                        lst = (i == len(kts) - 1)
                                        nc.tensor.matmul(
                                            pz[:], oneskr[:], e[:],
                                            start=fst, stop=lst,
                                            skip_group_check=True)
                                        nc.tensor.matmul(
                                            po[:],
                                            vnat[:, kt * 128:
                                                 (kt + 1) * 128], e[:],
                                            start=fst, stop=lst,
                                            skip_group_check=True)
                                    zr = zpool.tile([1, 512], F32R,
                                                    name=f"zr{b}{h}{qc}",
                                                    tag="zr")
                                    nc.vector.reciprocal(zr[:], pz[:])
                                    pzb = ps_x.tile([128, 512], F32,
                                                    name=f"pzb{b}{h}{qc}",
                                                    tag="pzb")
                                    nc.tensor.matmul(pzb[:], onesmr[:],
                                                     zr[:], start=True,
                                                     stop=True,
                                                     skip_group_check=True)
                                    zb = zpool.tile([128, 512], F32,
                                                    name=f"zb{b}{h}{qc}",
                                                    tag="zb")
                                    nc.scalar.copy(zb[:], pzb[:])
                                    ao = aop.tile([128, 512], F32,
                                                  name=f"ao{b}{h}{qc}",
                                                  tag="ao")
                                    nc.vector.tensor_mul(ao[:], po[:],
                                                         zb[:])
                                    nc.sync.dma_start(
                                        out=agin[b][h * 128:(h + 1) * 128,
                                                    qc * 512:
                                                    (qc + 1) * 512],
                                        in_=ao[:])
                            nc.gpsimd.collective_compute(
                                "AllGather", ALU.bypass, replica_groups=RG,
                                ins=[agin[b][:].opt()],
                                outs=[agout[b][:].opt()])

                # ---- phase 3: o_proj (out in natural [token, m] layout) ----
                with tc.tile_pool(name="a3", bufs=1) as a3, \
                     tc.tile_pool(name="o3", bufs=2) as o3, \
                     tc.tile_pool(name="p3", bufs=4, space="PSUM") as p3:
                    for ch in range(NTC):
                        b, q2 = ch // 2, ch % 2
                        ats = []
                        for kt in range(NKT):
                            at = a3.tile([128, 512], F32R,
                                         name=f"at{ch}_{kt}", tag=f"at{kt}")
                            nc.sync.dma_start(
                                out=at[:],
                                in_=agout[b][kt * 128:(kt + 1) * 128,
                                             q2 * 512:(q2 + 1) * 512]
                                .bitcast(F32R))
                            ats.append(at)
                        for tt in range(4):
                            pout = p3.tile([128, 512], F32,
                                           name=f"po3_{ch}{tt}", tag="pout")
                            for kt in range(NKT):
                                nc.tensor.matmul(
                                    pout[:],
                                    ats[kt][:, tt * 128:(tt + 1) * 128],
                                    wo_sb[:, kt * MSH:(kt + 1) * MSH],
                                    start=(kt == 0), stop=(kt == NKT - 1),
                                    skip_group_check=True)
                            osb = o3.tile([128, 512], F32,
                                          name=f"osb{ch}{tt}", tag="osb")
                            nc.scalar.copy(osb[:], pout[:])
                            nc.sync.dma_start(
                                out=outN[ch * 512 + tt * 128:
                                         ch * 512 + (tt + 1) * 128, :],
                                in_=osb[:])
                w3.release()


    nc.compile()
    return nc


def kernel(hidden_states, Wq, Wk, Wv, Wo, attention_mask, position_ids):
    from concourse.bass_utils import run_bass_kernel_spmd
    from concourse.bass_interp import get_hw_module

    hs = np.ascontiguousarray(np.asarray(hidden_states, dtype=np.float32))
    Wq = np.asarray(Wq, dtype=np.float32)
    Wk = np.asarray(Wk, dtype=np.float32)
    Wv = np.asarray(Wv, dtype=np.float32)
    Wo = np.asarray(Wo, dtype=np.float32)
    mask = np.asarray(attention_mask, dtype=np.float32)
    posf = np.ascontiguousarray(
        np.asarray(position_ids).reshape(1, T).astype(np.float32))

    status, index, packed = _classify_mask(mask)
    n_packed = packed.shape[0] // 128

    key = (status.tobytes(), n_packed)
    if key not in _cache:
        nc = _build(status, index, n_packed)
        nc.m = get_hw_module(nc.m)
        _cache[key] = nc
    nc = _cache[key]

    xT = np.ascontiguousarray(hs.reshape(T, H).T)
    in_maps = []
    for c in range(NCORES):
        in_maps.append({
            "xT": xT,
            "wqT": np.ascontiguousarray(
                Wq[c * QH * HD:(c + 1) * QH * HD, :].T),
            "wkT": np.ascontiguousarray(Wk[c * HD:(c + 1) * HD, :].T),
            "wvT": np.ascontiguousarray(Wv[c * HD:(c + 1) * HD, :].T),
            "woT": np.ascontiguousarray(Wo[c * MSH:(c + 1) * MSH, :].T),
            "maskP": packed,
            "pos": posf,
        })
    res = run_bass_kernel_spmd(nc, in_maps, core_ids=list(range(NCORES)),
                               trace=bool(os.environ.get("BITNET_TRACE")))
    global last_exec_time_ns
    last_exec_time_ns = res.exec_time_ns
    out = np.concatenate(
        [res.results[c]["outN"] for c in range(NCORES)], axis=1)  # (T, H)
    return np.ascontiguousarray(out).reshape(B, S, H).astype(np.float32)



# revision 6
# speedup vs baseline: 1.2431x; 1.2431x over previous
"""BitNet attention block on 8 TRN2 NeuronCores (tensor-parallel over heads).

Self-contained: kernel(**inputs) takes full inputs, shards internally,
runs one SPMD Bass program on cores 0-7, reassembles the full output.

Sharding: core c owns Q heads [4c,4c+4), KV head c, o_proj output dims
[512c, 512c+512). Attention is fully local per core. Cross-core comms:
two tiny AllReduces for the BitNet absmean gammas and two bf16
AllGathers (one per batch) of the attention output.

Optimization notes vs the fp32r baseline:
- Weights are quantized to EXACT ternary {-1,0,+1} stored as bf16; the
  gamma scale factors are folded into activation `scale=` APs instead
  (exp scale carries gq*gk/sqrt(HD), the V psum->sbuf copy carries gv,
  the o_proj psum->sbuf copy carries go). x is converted to bf16 on the
  host. Quantization decisions (round-half-even + clip) are reproduced
  exactly with a 2-op compare chain: wq = (w > g/2) - (w < -g/2).
- All four weight gamma prepasses run up front (Wk/Wv staged in fp32
  SBUF, Wq/Wo streamed); the two gamma AllReduces are issued
  back-to-back so their ~30us floors overlap the Wq re-read. Wo's
  quant is interleaved into phase 1 where the Vector engine has slack,
  and the quantized Wo stays resident in SBUF (no DRAM bounce).
- Distinct mask blocks are deduped host-side (causal -> 4 blocks) and
  kept SBUF-resident across batches/heads.
- AllGather payloads are bf16 (half the bytes); phase 3 for batch b
  overlaps the other batch's AllGather / attention.
"""
import os
import sys
sys.path.insert(0, "/opt/trn_rl_repo")
import numpy as np
import ml_dtypes

B, S, H = 2, 1024, 4096
NH, NKV, HD = 32, 8, 128
NCORES = 8
T = B * S
QH = NH // NCORES          # 4 q-heads per core
MSH = H // NCORES          # 512 o_proj out-dims per core
THETA = 10000.0
C_MAGIC = 12582912.0       # 1.5 * 2**23
TWO_PI = 6.283185307179586
NKT = H // 128             # 32 contraction tiles
NTC = T // 512             # 4 token chunks
SKT = S // 128             # 8 score k-tiles per batch
SQC = S // 512             # 2 q-chunks per batch
BF16NP = ml_dtypes.bfloat16

_cache = {}
last_exec_time_ns = None


def _classify_mask(mask):
    """Per (b, kt, qc) [128k x 512q] block: 0 no-op, 1 fully masked
    (skipped), 2 needs a mask add (index into deduped distinct blocks)."""
    status = np.empty((B, SKT, SQC), dtype=np.int8)
    blk_idx = {}
    distinct = []
    seen = {}
    for b in range(B):
        mb = np.asarray(mask[b, 0], dtype=np.float32)
        for kt in range(SKT):
            for qc in range(SQC):
                blk = mb[qc * 512:(qc + 1) * 512, kt * 128:(kt + 1) * 128]
                if not blk.any():
                    status[b, kt, qc] = 0
                elif (blk <= -1e4).all():
                    status[b, kt, qc] = 1
                else:
                    status[b, kt, qc] = 2
                    kb = blk.tobytes()
                    if kb not in seen:
                        seen[kb] = len(distinct)
                        distinct.append(np.ascontiguousarray(blk.T))
                    blk_idx[(b, kt, qc)] = seen[kb]
    if distinct:
        packed = np.concatenate(distinct, axis=0).astype(np.float32)
    else:
        packed = np.zeros((128, 512), dtype=np.float32)
    return status, blk_idx, packed


def _cody_consts():
    c1 = float(np.float32(6.28125))
    r = np.float64(TWO_PI) - c1
    c2 = float(np.float32(r - np.remainder(r, 2.0 ** -24)))
    c3 = float(np.float32(np.float64(TWO_PI) - c1 - float(c2)))
    return c1, c2, c3


def _build(status, blk_idx, n_blk):
    from concourse import bacc, tile, mybir

    F32 = mybir.dt.float32
    F32R = mybir.dt.float32r
    BF = mybir.dt.bfloat16
    ACTF = mybir.ActivationFunctionType
    ALU = mybir.AluOpType
    X = mybir.AxisListType.X
    RG = [list(range(NCORES))]
    c1, c2, c3 = _cody_consts()

    nc = bacc.Bacc("TRN2", target_bir_lowering=False, debug=False,
                   num_devices=NCORES)

    xT = nc.dram_tensor("xT", [H, T], BF, kind="ExternalInput")
    wqT = nc.dram_tensor("wqT", [H, QH * HD], F32, kind="ExternalInput")
    wkT = nc.dram_tensor("wkT", [H, HD], F32, kind="ExternalInput")
    wvT = nc.dram_tensor("wvT", [H, HD], F32, kind="ExternalInput")
    woT = nc.dram_tensor("woT", [H, MSH], F32, kind="ExternalInput")
    maskP = nc.dram_tensor("maskP", [n_blk * 128, 512], F32,
                           kind="ExternalInput")
    pos = nc.dram_tensor("pos", [1, T], F32, kind="ExternalInput")
    outN = nc.dram_tensor("outN", [T, MSH], F32, kind="ExternalOutput")

    idn_c = nc.inline_tensor(np.eye(128, dtype=np.float32), name="idn_c")
    onesk_c = nc.inline_tensor(np.ones((128, 1), np.float32), name="onesk_c")
    onesm_c = nc.inline_tensor(np.ones((1, 128), np.float32), name="onesm_c")
    invf_np = (1.0 / THETA ** (np.arange(0, HD, 2, dtype=np.float32) / HD))
    invf_np = np.concatenate([invf_np, invf_np]).reshape(HD, 1)
    invf_c = nc.inline_tensor(invf_np.astype(np.float32), name="invf_c")

    NQ = float(NH * HD * H)
    NK = float(NKV * HD * H)
    NO = float(H * NH * HD)
    ISQ = float(1.0 / np.sqrt(HD))

    with tile.TileContext(nc) as tc, \
         nc.allow_low_precision(reason="bf16 ternary kernel"):
        with tc.tile_pool(name="cpool", bufs=1) as cpool, \
             tc.tile_pool(name="dbounce", bufs=1, space="DRAM") as dbounce:
            # DRAM bounce tiles for the collectives
            arq_in = dbounce.tile([1, 8], F32, name="arq_in")
            arq_out = dbounce.tile([1, 8], F32, name="arq_out",
                                   addr_space="Shared")
            aro_in = dbounce.tile([1, 8], F32, name="aro_in")
            aro_out = dbounce.tile([1, 8], F32, name="aro_out",
                                   addr_space="Shared")
            agin = [dbounce.tile([QH * HD, S], BF, name=f"agin{b}")
                    for b in range(B)]
            agout = [dbounce.tile([H, S], BF, name=f"agout{b}",
                                  addr_space="Shared") for b in range(B)]

            # constants
            idn = cpool.tile([128, 128], F32R, name="idn")
            nc.sync.dma_start(out=idn[:], in_=idn_c[:, :].bitcast(F32R))
            oneskr = cpool.tile([128, 1], F32R, name="oneskr")
            nc.sync.dma_start(out=oneskr[:], in_=onesk_c[:, :].bitcast(F32R))
            onesk = cpool.tile([128, 1], F32, name="onesk")
            nc.sync.dma_start(out=onesk[:], in_=onesk_c[:, :])
            onesmr = cpool.tile([1, 128], F32R, name="onesmr")
            nc.sync.dma_start(out=onesmr[:], in_=onesm_c[:, :].bitcast(F32R))
            onesm = cpool.tile([1, 128], F32, name="onesm")
            nc.sync.dma_start(out=onesm[:], in_=onesm_c[:, :])
            invf = cpool.tile([128, 1], F32, name="invf")
            nc.sync.dma_start(out=invf[:], in_=invf_c[:, :])
            mask_sb = cpool.tile([128, n_blk * 512], F32, name="mask_sb")
            for i in range(n_blk):
                nc.sync.dma_start(out=mask_sb[:, i * 512:(i + 1) * 512],
                                  in_=maskP[i * 128:(i + 1) * 128, :])
            # broadcast scalars: 0 thq 1 thqn 2 thk 3 thkn 4 thv 5 thvn
            #                    6 cqk 7 gv | 8 tho 9 thon 10 go
            bsc = cpool.tile([128, 12], F32, name="bsc")

            with tc.tile_pool(name="tab", bufs=1) as tab:
                cos_sb = tab.tile([128, T], F32, name="cos_sb")
                ss_sb = tab.tile([128, T], F32, name="ss_sb")
                # RoPE tables: Cody-Waite range reduction + Sin (emitted
                # first so they run during the initial weight DMA).
                with tc.tile_pool(name="rtab", bufs=3) as rtab, \
                     tc.tile_pool(name="prt", bufs=2, space="PSUM") as prt:
                    for tcn in range(NTC):
                        cs = slice(tcn * 512, (tcn + 1) * 512)
                        pchunk = rtab.tile([1, 512], F32, name=f"pos{tcn}",
                                           tag="pos")
                        nc.sync.dma_start(out=pchunk[:], in_=pos[0:1, cs])
                        pf = prt.tile([128, 512], F32, name=f"pf{tcn}",
                                      tag="pf")
                        nc.tensor.matmul(pf[:], onesm[:], pchunk[:],
                                         start=True, stop=True)
                        f_sb = rtab.tile([128, 512], F32, name=f"f{tcn}",
                                         tag="f")
                        nc.scalar.activation(f_sb[:], pf[:], ACTF.Copy,
                                             scale=invf[:])
                        k_sb = rtab.tile([128, 512], F32, name=f"kk{tcn}",
                                         tag="kk")
                        nc.vector.tensor_scalar(k_sb[:], f_sb[:],
                                                1.0 / TWO_PI, C_MAGIC,
                                                ALU.mult, ALU.add)
                        nc.vector.tensor_scalar(k_sb[:], k_sb[:], C_MAGIC,
                                                None, ALU.subtract)
                        y_sb = rtab.tile([128, 512], F32, name=f"y{tcn}",
                                         tag="y")
                        nc.vector.scalar_tensor_tensor(
                            y_sb[:], k_sb[:], -c1, f_sb[:], ALU.mult,
                            ALU.add)
                        nc.vector.scalar_tensor_tensor(
                            y_sb[:], k_sb[:], -c2, y_sb[:], ALU.mult,
                            ALU.add)
                        nc.vector.scalar_tensor_tensor(
                            y_sb[:], k_sb[:], -c3, y_sb[:], ALU.mult,
                            ALU.add)
                        nc.scalar.activation(ss_sb[0:64, cs], y_sb[0:64, :],
                                             ACTF.Sin, scale=-1.0)
                        nc.scalar.activation(ss_sb[64:128, cs],
                                             y_sb[64:128, :], ACTF.Sin)
                        yc = rtab.tile([128, 512], F32, name=f"yc{tcn}",
                                       tag="yc")
                        nc.vector.tensor_scalar(yc[:], y_sb[:],
                                                float(np.pi / 2), None,
                                                ALU.add)
                        m_sb = rtab.tile([128, 512], F32, name=f"mm{tcn}",
                                         tag="mm")
                        nc.vector.tensor_scalar(m_sb[:], yc[:],
                                                float(np.pi), None,
                                                ALU.is_gt)
                        nc.vector.scalar_tensor_tensor(
                            yc[:], m_sb[:], -TWO_PI, yc[:], ALU.mult,
                            ALU.add)
                        nc.scalar.activation(cos_sb[:, cs], yc[:], ACTF.Sin)

                # ---- pools ordered by lifetime for LIFO release ----
                gacc = tc.alloc_tile_pool(name="gacc", bufs=1)
                w3 = tc.alloc_tile_pool(name="w3", bufs=1)
                wo_sb = w3.tile([128, NKT * MSH], BF, name="wo_sb")
                qkvp = tc.alloc_tile_pool(name="qkv", bufs=1)
                qT_sb = [qkvp.tile([128, T], BF, name=f"qT{h}")
                         for h in range(QH)]
                kT_sb = qkvp.tile([128, T], BF, name="kT_sb")
                vT_sb = qkvp.tile([128, T], F32R, name="vT_sb")
                wbig = tc.alloc_tile_pool(name="wbig", bufs=1)
                wq_sb = wbig.tile([128, NKT * 512], BF, name="wq_sb")
                wbig2 = tc.alloc_tile_pool(name="wbig2", bufs=1)
                wk_sb = wbig2.tile([128, NKT * 128], BF, name="wk_sb")
                wv_sb = wbig2.tile([128, NKT * 128], BF, name="wv_sb")

                # ---- gamma prepass: Wq/Wo streamed, Wk/Wv staged ----
                wstage = tc.alloc_tile_pool(name="wstage", bufs=1)
                wk_f = wstage.tile([128, NKT * 128], F32, name="wk_f")
                wv_f = wstage.tile([128, NKT * 128], F32, name="wv_f")

                accq = gacc.tile([128, NKT], F32, name="accq")
                acck = gacc.tile([128, NKT], F32, name="acck")
                accv = gacc.tile([128, NKT], F32, name="accv")
                acco = gacc.tile([128, NKT], F32, name="acco")
                g4 = gacc.tile([128, 4], F32, name="g4")

                with tc.tile_pool(name="wqpre", bufs=8) as wqpre, \
                     tc.tile_pool(name="wopre", bufs=8) as wopre:
                    for i in range(NKT):
                        sl = wqpre.tile([128, 512], F32, name=f"wqp{i}",
                                        tag="wqp")
                        nc.sync.dma_start(out=sl[:],
                                          in_=wqT[i * 128:(i + 1) * 128, :])
                        nc.vector.tensor_reduce(accq[:, i:i + 1], sl[:], X,
                                                ALU.add,
                                                apply_absolute_value=True)
                    for i in range(NKT):
                        sl = wk_f[:, i * 128:(i + 1) * 128]
                        nc.gpsimd.dma_start(
                            out=sl, in_=wkT[i * 128:(i + 1) * 128, :])
                        nc.vector.tensor_reduce(acck[:, i:i + 1], sl, X,
                                                ALU.add,
                                                apply_absolute_value=True)
                        sl = wv_f[:, i * 128:(i + 1) * 128]
                        nc.gpsimd.dma_start(
                            out=sl, in_=wvT[i * 128:(i + 1) * 128, :])
                        nc.vector.tensor_reduce(accv[:, i:i + 1], sl, X,
                                                ALU.add,
                                                apply_absolute_value=True)
                    # Wo |.| sums ride along up front too (vector is
                    # otherwise idle waiting for the AllReduce)
                    for i in range(NKT):
                        sl = wopre.tile([128, 512], F32, name=f"wop{i}",
                                        tag="wop")
                        nc.gpsimd.dma_start(
                            out=sl[:], in_=woT[i * 128:(i + 1) * 128, :])
                        nc.vector.tensor_reduce(acco[:, i:i + 1], sl[:], X,
                                                ALU.add,
                                                apply_absolute_value=True)
                    nc.vector.tensor_reduce(g4[:, 0:1], accq[:], X, ALU.add)
                    nc.vector.tensor_reduce(g4[:, 1:2], acck[:], X, ALU.add)
                    nc.vector.tensor_reduce(g4[:, 2:3], accv[:], X, ALU.add)
                    nc.vector.tensor_reduce(g4[:, 3:4], acco[:], X, ALU.add)

                with tc.tile_pool(name="pgam", bufs=2, space="PSUM") as pgam:
                    # AllReduce #1: q/k/v gamma sums
                    pg_q = pgam.tile([1, 3], F32, name="pg_q")
                    nc.tensor.matmul(pg_q[:], onesk[:], g4[:, 0:3],
                                     start=True, stop=True)
                    gq_sb = gacc.tile([1, 8], F32, name="gq_sb")
                    nc.vector.memset(gq_sb[:], 0.0)
                    nc.scalar.copy(gq_sb[:, 0:3], pg_q[:])
                    nc.sync.dma_start(out=arq_in[:], in_=gq_sb[:])
                    nc.gpsimd.collective_compute(
                        "AllReduce", ALU.add, replica_groups=RG,
                        ins=[arq_in[:].opt()], outs=[arq_out[:].opt()])
                    arq_sb = gacc.tile([1, 8], F32, name="arq_sb")
                    nc.sync.dma_start(out=arq_sb[:], in_=arq_out[:])

                    # AllReduce #2: Wo gamma sum (issued right behind #1)
                    pg_o = pgam.tile([1, 1], F32, name="pg_o")
                    nc.tensor.matmul(pg_o[:], onesk[:], g4[:, 3:4],
                                     start=True, stop=True)
                    go_sb = gacc.tile([1, 8], F32, name="go_sb")
                    nc.vector.memset(go_sb[:], 0.0)
                    nc.scalar.copy(go_sb[:, 0:1], pg_o[:])
                    nc.sync.dma_start(out=aro_in[:], in_=go_sb[:])
                    nc.gpsimd.collective_compute(
                        "AllReduce", ALU.add, replica_groups=RG,
                        ins=[aro_in[:].opt()], outs=[aro_out[:].opt()])
                    aro_sb = gacc.tile([1, 8], F32, name="aro_sb")
                    nc.sync.dma_start(out=aro_sb[:], in_=aro_out[:])

                    # q/k/v gammas -> thresholds / folded scales
                    gam = gacc.tile([1, 8], F32, name="gam")
                    nc.vector.tensor_scalar(gam[:, 0:1], arq_sb[:, 0:1],
                                            1.0 / NQ, 1e-5, ALU.mult,
                                            ALU.add)
                    nc.vector.tensor_scalar(gam[:, 1:3], arq_sb[:, 1:3],
                                            1.0 / NK, 1e-5, ALU.mult,
                                            ALU.add)
                    bcin = gacc.tile([1, 8], F32, name="bcin")
                    nc.vector.tensor_scalar(bcin[:, 0:1], gam[:, 0:1], 0.5,
                                            None, ALU.mult)
                    nc.vector.tensor_scalar(bcin[:, 1:2], gam[:, 0:1], -0.5,
                                            None, ALU.mult)
                    nc.vector.tensor_scalar(bcin[:, 2:3], gam[:, 1:2], 0.5,
                                            None, ALU.mult)
                    nc.vector.tensor_scalar(bcin[:, 3:4], gam[:, 1:2], -0.5,
                                            None, ALU.mult)
                    nc.vector.tensor_scalar(bcin[:, 4:5], gam[:, 2:3], 0.5,
                                            None, ALU.mult)
                    nc.vector.tensor_scalar(bcin[:, 5:6], gam[:, 2:3], -0.5,
                                            None, ALU.mult)
                    nc.vector.tensor_mul(bcin[:, 6:7], gam[:, 0:1],
                                         gam[:, 1:2])
                    nc.vector.tensor_scalar(bcin[:, 6:7], bcin[:, 6:7], ISQ,
                                            None, ALU.mult)
                    nc.vector.tensor_copy(bcin[:, 7:8], gam[:, 2:3])
                    pgb1 = pgam.tile([128, 8], F32, name="pgb1")
                    nc.tensor.matmul(pgb1[:], onesm[:], bcin[:], start=True,
                                     stop=True)
                    nc.scalar.copy(bsc[:, 0:8], pgb1[:])

                    THQ, THQN = bsc[:, 0:1], bsc[:, 1:2]
                    THK, THKN = bsc[:, 2:3], bsc[:, 3:4]
                    THV, THVN = bsc[:, 4:5], bsc[:, 5:6]
                    CQK, GV = bsc[:, 6:7], bsc[:, 7:8]
                    THO, THON, GO = bsc[:, 8:9], bsc[:, 9:10], bsc[:, 10:11]

                    # ---- quant q/k/v -> exact ternary bf16 (Wq re-read;
                    # interleaved per-kt so phase 1's PE can chase it)
                    with tc.tile_pool(name="wqst", bufs=10) as wqst, \
                         tc.tile_pool(name="qscr", bufs=4) as qscr:
                        def quant_tile(src, dst, thp, thn, tg):
                            scr = qscr.tile([128, src.shape[1]], F32,
                                            name=f"qs_{tg}",
                                            tag=f"qs{tg[0]}")
                            nc.vector.tensor_scalar(scr[:], src, thn, None,
                                                    ALU.is_lt)
                            nc.vector.scalar_tensor_tensor(
                                dst, src, thp, scr[:], ALU.is_gt,
                                ALU.subtract)

                        for i in range(NKT):
                            wq2 = wqst.tile([128, 512], F32,
                                            name=f"wq2_{i}", tag="wq2")
                            nc.sync.dma_start(
                                out=wq2[:],
                                in_=wqT[i * 128:(i + 1) * 128, :])
                            quant_tile(wq2[:],
                                       wq_sb[:, i * 512:(i + 1) * 512],
                                       THQ, THQN, f"q{i}")
                            quant_tile(wk_f[:, i * 128:(i + 1) * 128],
                                       wk_sb[:, i * 128:(i + 1) * 128],
                                       THK, THKN, f"k{i}")
                            quant_tile(wv_f[:, i * 128:(i + 1) * 128],
                                       wv_sb[:, i * 128:(i + 1) * 128],
                                       THV, THVN, f"v{i}")

                    # Wo gamma -> thresholds (after quant emission so the
                    # vector engine doesn't stall on AllReduce #2)
                    gamo = gacc.tile([1, 3], F32, name="gamo")
                    nc.vector.tensor_scalar(gamo[:, 2:3], aro_sb[:, 0:1],
                                            1.0 / NO, 1e-5, ALU.mult,
                                            ALU.add)
                    nc.vector.tensor_scalar(gamo[:, 0:1], gamo[:, 2:3], 0.5,
                                            None, ALU.mult)
                    nc.vector.tensor_scalar(gamo[:, 1:2], gamo[:, 2:3], -0.5,
                                            None, ALU.mult)
                    pgb2 = pgam.tile([128, 3], F32, name="pgb2")
                    nc.tensor.matmul(pgb2[:], onesm[:], gamo[:], start=True,
                                     stop=True)
                    nc.scalar.copy(bsc[:, 8:11], pgb2[:])
                wstage.release()

                # ---- phase 1: QKV projections + RoPE + Wo quant ----
                OPT = NKT // NTC  # wo quant tiles per token chunk
                with tc.tile_pool(name="xin", bufs=6) as xin, \
                     tc.tile_pool(name="rope", bufs=2) as rope, \
                     tc.tile_pool(name="wop2", bufs=6) as wop2, \
                     tc.tile_pool(name="qsc2", bufs=4) as qsc2, \
                     tc.tile_pool(name="p1", bufs=8, space="PSUM") as p1:
                    for tcn in range(NTC):
                        cs = slice(tcn * 512, (tcn + 1) * 512)
                        pq = [p1.tile([128, 512], F32, name=f"pq{tcn}_{h}",
                                      tag="p1") for h in range(QH)]
                        pk = p1.tile([128, 512], F32, name=f"pk{tcn}",
                                     tag="p1")
                        pv = p1.tile([128, 512], F32, name=f"pv{tcn}",
                                     tag="p1")
                        for kt in range(NKT):
                            xt = xin.tile([128, 512], BF,
                                          name=f"x{tcn}_{kt}", tag="xt")
                            nc.sync.dma_start(
                                out=xt[:],
                                in_=xT[kt * 128:(kt + 1) * 128, cs])
                            st, sp = (kt == 0), (kt == NKT - 1)
                            for h in range(QH):
                                nc.tensor.matmul(
                                    pq[h][:],
                                    wq_sb[:, kt * 512 + h * 128:
                                          kt * 512 + (h + 1) * 128],
                                    xt[:], start=st, stop=sp,
                                    skip_group_check=True)
                            nc.tensor.matmul(
                                pk[:], wk_sb[:, kt * 128:(kt + 1) * 128],
                                xt[:], start=st, stop=sp,
                                skip_group_check=True)
                            nc.tensor.matmul(
                                pv[:], wv_sb[:, kt * 128:(kt + 1) * 128],
                                xt[:], start=st, stop=sp,
                                skip_group_check=True)

                        def rope_apply(psrc, dst_ap, tg):
                            m1 = rope.tile([128, 512], F32, name=f"m1{tg}",
                                           tag="m1")
                            nc.vector.tensor_mul(m1[:], psrc[:],
                                                 cos_sb[:, cs])
                            m2 = rope.tile([128, 512], F32, name=f"m2{tg}",
                                           tag="m2")
                            nc.vector.tensor_mul(m2[0:64, :],
                                                 psrc[64:128, :],
                                                 ss_sb[0:64, cs])
                            nc.vector.tensor_mul(m2[64:128, :],
                                                 psrc[0:64, :],
                                                 ss_sb[64:128, cs])
                            nc.vector.tensor_add(dst_ap, m1[:], m2[:])

                        for h in range(QH):
                            rope_apply(pq[h], qT_sb[h][:, cs], f"_{tcn}_{h}")
                        rope_apply(pk, kT_sb[:, cs], f"k_{tcn}")
                        nc.scalar.activation(vT_sb[:, cs], pv[:], ACTF.Copy,
                                             scale=GV)

                        # Wo quant rides along in the vector slack
                        for j in range(OPT):
                            i = tcn * OPT + j
                            wt = wop2.tile([128, 512], F32, name=f"wo2_{i}",
                                           tag="wo2")
                            nc.sync.dma_start(
                                out=wt[:], in_=woT[i * 128:(i + 1) * 128, :])
                            scr = qsc2.tile([128, 512], F32, name=f"os{i}",
                                            tag="os")
                            nc.vector.tensor_scalar(scr[:], wt[:], THON,
                                                    None, ALU.is_lt)
                            nc.vector.scalar_tensor_tensor(
                                wo_sb[:, i * MSH:(i + 1) * MSH], wt[:], THO,
                                scr[:], ALU.is_gt, ALU.subtract)

                wbig2.release()
                wbig.release()

                # ---- phase 2: attention ----
                with tc.tile_pool(name="vnatp", bufs=2) as vnatp, \
                     tc.tile_pool(name="epool", bufs=6) as epool, \
                     tc.tile_pool(name="aop", bufs=4) as aop, \
                     tc.tile_pool(name="zpool", bufs=2) as zpool, \
                     tc.tile_pool(name="ps_s", bufs=2,
                                  space="PSUM") as ps_s, \
                     tc.tile_pool(name="ps_o", bufs=2,
                                  space="PSUM") as ps_o, \
                     tc.tile_pool(name="ps_x", bufs=1,
                                  space="PSUM") as ps_x:
                    for b in range(B):
                        boff = b * S
                        vnat = vnatp.tile([128, S], F32R, name=f"vnat{b}",
                                          tag="vnat")
                        for kt in range(SKT):
                            ptr = ps_x.tile([128, 128], F32R,
                                            name=f"ptr{b}_{kt}", tag="ptr")
                            nc.tensor.transpose(
                                ptr[:],
                                vT_sb[:, boff + kt * 128:
                                      boff + (kt + 1) * 128], idn[:])
                            nc.scalar.copy(
                                vnat[:, kt * 128:(kt + 1) * 128],
                                ptr[:].bitcast(F32))
                        for qc in range(SQC):
                            kts = [kt for kt in range(SKT)
                                   if status[b, kt, qc] != 1]
                            assert kts, "fully-masked softmax row"
                            for h in range(QH):
                                qsl = qT_sb[h][:, boff + qc * 512:
                                               boff + (qc + 1) * 512]
                                pz = ps_x.tile([1, 512], F32,
                                               name=f"pz{b}{h}{qc}",
                                               tag="pz")
                                po = ps_o.tile([128, 512], F32,
                                               name=f"po{b}{h}{qc}",
                                               tag="po")
                                for i, kt in enumerate(kts):
                                    ps_ = ps_s.tile([128, 512], F32,
                                                    name=f"s{b}{h}{qc}{kt}",
                                                    tag="ps")
                                    nc.tensor.matmul(
                                        ps_[:],
                                        kT_sb[:, boff + kt * 128:
                                              boff + (kt + 1) * 128],
                                        qsl, start=True, stop=True,
                                        skip_group_check=True)
                                    if status[b, kt, qc] == 2:
                                        mi = blk_idx[(b, kt, qc)]
                                        nc.vector.tensor_add(
                                            ps_[:], ps_[:],
                                            mask_sb[:, mi * 512:
                                                    (mi + 1) * 512])
                                    e = epool.tile([128, 512], F32R,
                                                   name=f"e{b}{h}{qc}{kt}",
                                                   tag="e")
                                    nc.scalar.activation(e[:], ps_[:],
                                                         ACTF.Exp,
                                                         scale=CQK)
                                    fst = (i == 0)
                                    lst = (i == len(kts) - 1)
                                    nc.tensor.matmul(
                                        pz[:], oneskr[:], e[:], start=fst,
                                        stop=lst, skip_group_check=True)
                                    nc.tensor.matmul(
                                        po[:],
                                        vnat[:, kt * 128:(kt + 1) * 128],
                                        e[:], start=fst, stop=lst,
                                        skip_group_check=True)
                                zr = zpool.tile([1, 512], F32R,
                                                name=f"zr{b}{h}{qc}",
                                                tag="zr")
                                nc.vector.reciprocal(zr[:], pz[:])
                                pzb = ps_x.tile([128, 512], F32,
                                                name=f"pzb{b}{h}{qc}",
                                                tag="pzb")
                                nc.tensor.matmul(pzb[:], onesmr[:], zr[:],
                                                 start=True, stop=True,
                                                 skip_group_check=True)
                                zb = zpool.tile([128, 512], F32,
                                                name=f"zb{b}{h}{qc}",
                                                tag="zb")
                                nc.scalar.copy(zb[:], pzb[:])
                                ao = aop.tile([128, 512], BF,
                                              name=f"ao{b}{h}{qc}",
                                              tag="ao")
                                nc.vector.tensor_mul(ao[:], po[:], zb[:])
                                nc.sync.dma_start(
                                    out=agin[b][h * 128:(h + 1) * 128,
                                                qc * 512:(qc + 1) * 512],
                                    in_=ao[:])
                        nc.gpsimd.collective_compute(
                            "AllGather", ALU.bypass, replica_groups=RG,
                            ins=[agin[b][:].opt()],
                            outs=[agout[b][:].opt()])

                # ---- phase 3: o_proj, split per batch for AG overlap ----
                with tc.tile_pool(name="a3", bufs=1) as a3, \
                     tc.tile_pool(name="o3", bufs=2) as o3, \
                     tc.tile_pool(name="p3", bufs=4, space="PSUM") as p3:
                    for ch in range(NTC):
                        b, q2 = ch // 2, ch % 2
                        ats = []
                        for kt in range(NKT):
                            at = a3.tile([128, 512], BF, name=f"at{ch}_{kt}",
                                         tag=f"at{kt}")
                            nc.sync.dma_start(
                                out=at[:],
                                in_=agout[b][kt * 128:(kt + 1) * 128,
                                             q2 * 512:(q2 + 1) * 512])
                            ats.append(at)
                        for tt in range(4):
                            pout = p3.tile([128, 512], F32,
                                           name=f"po3_{ch}{tt}", tag="pout")
                            for kt in range(NKT):
                                nc.tensor.matmul(
                                    pout[:],
                                    ats[kt][:, tt * 128:(tt + 1) * 128],
                                    wo_sb[:, kt * MSH:(kt + 1) * MSH],
                                    start=(kt == 0), stop=(kt == NKT - 1),
                                    skip_group_check=True)
                            osb = o3.tile([128, 512], F32,
                                          name=f"osb{ch}{tt}", tag="osb")
                            nc.scalar.activation(osb[:], pout[:], ACTF.Copy,
                                                 scale=GO)
                            nc.sync.dma_start(
                                out=outN[ch * 512 + tt * 128:
                                         ch * 512 + (tt + 1) * 128, :],
                                in_=osb[:])
                qkvp.release()
                w3.release()
                gacc.release()

    nc.compile()
    return nc


def kernel(hidden_states, Wq, Wk, Wv, Wo, attention_mask, position_ids):
    from concourse.bass_utils import run_bass_kernel_spmd
    from concourse.bass_interp import get_hw_module

    hs = np.ascontiguousarray(np.asarray(hidden_states, dtype=np.float32))
    Wq = np.asarray(Wq, dtype=np.float32)
    Wk = np.asarray(Wk, dtype=np.float32)
    Wv = np.asarray(Wv, dtype=np.float32)
    Wo = np.asarray(Wo, dtype=np.float32)
    mask = np.asarray(attention_mask, dtype=np.float32)
    posf = np.ascontiguousarray(
        np.asarray(position_ids).reshape(1, T).astype(np.float32))

    status, blk_idx, packed = _classify_mask(mask)
    n_blk = packed.shape[0] // 128
    assert n_blk <= 16, "too many distinct mask blocks"

    key = (status.tobytes(), tuple(sorted(blk_idx.items())), n_blk)
    if key not in _cache:
        nc = _build(status, blk_idx, n_blk)
        nc.m = get_hw_module(nc.m)
        _cache[key] = nc
    nc = _cache[key]

    xT = np.ascontiguousarray(hs.reshape(T, H).T.astype(BF16NP))
    in_maps = []
    for c in range(NCORES):
        in_maps.append({
            "xT": xT,
            "wqT": np.ascontiguousarray(
                Wq[c * QH * HD:(c + 1) * QH * HD, :].T),
            "wkT": np.ascontiguousarray(Wk[c * HD:(c + 1) * HD, :].T),
            "wvT": np.ascontiguousarray(Wv[c * HD:(c + 1) * HD, :].T),
            "woT": np.ascontiguousarray(Wo[c * MSH:(c + 1) * MSH, :].T),
            "maskP": packed,
            "pos": posf,
        })
    res = run_bass_kernel_spmd(nc, in_maps, core_ids=list(range(NCORES)),
                               trace=bool(os.environ.get("BITNET_TRACE")))
    global last_exec_time_ns
    last_exec_time_ns = res.exec_time_ns
    out = np.concatenate(
        [res.results[c]["outN"] for c in range(NCORES)], axis=1)  # (T, MSH*8)
    return np.ascontiguousarray(out).reshape(B, S, H).astype(np.float32)


# revision 12
# speedup vs baseline: 1.2774x; 1.0276x over previous
"""BitNet attention block on 8 TRN2 NeuronCores (tensor-parallel over heads).

Self-contained: kernel(**inputs) takes full inputs, shards internally,
runs one SPMD Bass program on cores 0-7, reassembles the full output.

Sharding: core c owns Q heads [4c,4c+4), KV head c, o_proj output dims
[512c, 512c+512). Attention is fully local per core. Cross-core comms:
two tiny AllReduces for the BitNet absmean gammas and two bf16
AllGathers (one per batch) of the attention output.

Optimization notes vs the fp32r baseline:
- Weights are quantized to EXACT ternary {-1,0,+1} stored as bf16; the
  gamma scale factors are folded into activation `scale=` APs instead
  (exp scale carries gq*gk/sqrt(HD), the V psum->sbuf copy carries gv,
  the o_proj psum->sbuf copy carries go). x is converted to bf16 on the
  host. Quantization decisions (round-half-even + clip) are reproduced
  exactly with a 2-op compare chain: wq = (w > g/2) - (w < -g/2).
- DMA descriptor issue costs ~0.6us each, so tile loads are batched
  into multi-tile slab DMAs via AP rearrange (weights: 1-4 descriptors
  per tensor; x: 4 per token chunk; o_proj activations: 1 per chunk).
- The two gamma AllReduces are issued back-to-back up front; their
  ~30us floors overlap the Wq re-read and x prefetch (the AllReduce
  result reads live on the Scalar queue so the Sync DMA queue never
  blocks). All AR2-dependent work (Wo thresholds + quant) is emitted
  mid-phase-1 where the engines have slack. Quantized Wo stays
  resident in SBUF (no DRAM bounce).
- Phase 2 emits all score matmuls of a head before the pz/po
  accumulation matmuls so the PE never stalls on the mask+exp chain.
- AllGather payloads are bf16; phase 3 for batch b overlaps the other
  batch's AllGather / attention.
"""
import os
import sys
sys.path.insert(0, "/opt/trn_rl_repo")
import numpy as np
import ml_dtypes

B, S, H = 2, 1024, 4096
NH, NKV, HD = 32, 8, 128
NCORES = 8
T = B * S
QH = NH // NCORES          # 4 q-heads per core
MSH = H // NCORES          # 512 o_proj out-dims per core
THETA = 10000.0
C_MAGIC = 12582912.0       # 1.5 * 2**23
TWO_PI = 6.283185307179586
NKT = H // 128             # 32 contraction tiles
NTC = T // 512             # 4 token chunks
SKT = S // 128             # 8 score k-tiles per batch
SQC = S // 512             # 2 q-chunks per batch
BF16NP = ml_dtypes.bfloat16

_cache = {}
last_exec_time_ns = None


def _classify_mask(mask):
    """Per (b, kt, qc) [128k x 512q] block: 0 no-op, 1 fully masked
    (skipped), 2 needs a mask add (index into deduped distinct blocks)."""
    status = np.empty((B, SKT, SQC), dtype=np.int8)
    blk_idx = {}
    distinct = []
    seen = {}
    for b in range(B):
        mb = np.asarray(mask[b, 0], dtype=np.float32)
        for kt in range(SKT):
            for qc in range(SQC):
                blk = mb[qc * 512:(qc + 1) * 512, kt * 128:(kt + 1) * 128]
                if not blk.any():
                    status[b, kt, qc] = 0
                elif (blk <= -1e4).all():
                    status[b, kt, qc] = 1
                else:
                    status[b, kt, qc] = 2
                    kb = blk.tobytes()
                    if kb not in seen:
                        seen[kb] = len(distinct)
                        distinct.append(np.ascontiguousarray(blk.T))
                    blk_idx[(b, kt, qc)] = seen[kb]
    if distinct:
        packed = np.concatenate(distinct, axis=0).astype(np.float32)
    else:
        packed = np.zeros((128, 512), dtype=np.float32)
    return status, blk_idx, packed


def _cody_consts():
    c1 = float(np.float32(6.28125))
    r = np.float64(TWO_PI) - c1
    c2 = float(np.float32(r - np.remainder(r, 2.0 ** -24)))
    c3 = float(np.float32(np.float64(TWO_PI) - c1 - float(c2)))
    return c1, c2, c3


def _build(status, blk_idx, n_blk):
    from concourse import bacc, tile, mybir

    F32 = mybir.dt.float32
    F32R = mybir.dt.float32r
    BF = mybir.dt.bfloat16
    ACTF = mybir.ActivationFunctionType
    ALU = mybir.AluOpType
    X = mybir.AxisListType.X
    RG = [list(range(NCORES))]
    c1, c2, c3 = _cody_consts()

    nc = bacc.Bacc("TRN2", target_bir_lowering=False, debug=False,
                   num_devices=NCORES)

    xT = nc.dram_tensor("xT", [H, T], BF, kind="ExternalInput")
    wqT = nc.dram_tensor("wqT", [H, QH * HD], F32, kind="ExternalInput")
    wkT = nc.dram_tensor("wkT", [H, HD], F32, kind="ExternalInput")
    wvT = nc.dram_tensor("wvT", [H, HD], F32, kind="ExternalInput")
    woT = nc.dram_tensor("woT", [H, MSH], F32, kind="ExternalInput")
    maskP = nc.dram_tensor("maskP", [n_blk * 128, 512], F32,
                           kind="ExternalInput")
    pos = nc.dram_tensor("pos", [1, T], F32, kind="ExternalInput")
    outN = nc.dram_tensor("outN", [T, MSH], F32, kind="ExternalOutput")

    idn_c = nc.inline_tensor(np.eye(128, dtype=np.float32), name="idn_c")
    onesk_c = nc.inline_tensor(np.ones((128, 1), np.float32), name="onesk_c")
    onesm_c = nc.inline_tensor(np.ones((1, 128), np.float32), name="onesm_c")
    invf_np = (1.0 / THETA ** (np.arange(0, HD, 2, dtype=np.float32) / HD))
    invf_np = np.concatenate([invf_np, invf_np]).reshape(HD, 1)
    invf_c = nc.inline_tensor(invf_np.astype(np.float32), name="invf_c")

    NQ = float(NH * HD * H)
    NK = float(NKV * HD * H)
    NO = float(H * NH * HD)
    ISQ = float(1.0 / np.sqrt(HD))

    def tiled(src):
        """[(i 128), c] DRAM slice -> [128, i, c] AP (partition-major)."""
        return src.rearrange("(i p) c -> i p c", p=128).transpose([1, 0, 2])

    def slab(t, j, w, cols=None):
        """DRAM rows [j*1024,(j+1)*1024) x w cols -> [128, 8, w] AP."""
        src = t[j * 1024:(j + 1) * 1024, :] if cols is None else \
            t[j * 1024:(j + 1) * 1024, cols]
        return tiled(src)

    with tile.TileContext(nc) as tc, \
         nc.allow_low_precision(reason="bf16 ternary kernel"):
        with tc.tile_pool(name="cpool", bufs=1) as cpool, \
             tc.tile_pool(name="dbounce", bufs=1, space="DRAM") as dbounce:
            # DRAM bounce tiles for the collectives
            arq_in = dbounce.tile([1, 8], F32, name="arq_in")
            arq_out = dbounce.tile([1, 8], F32, name="arq_out",
                                   addr_space="Shared")
            aro_in = dbounce.tile([1, 8], F32, name="aro_in")
            aro_out = dbounce.tile([1, 8], F32, name="aro_out",
                                   addr_space="Shared")
            agin = [dbounce.tile([QH * HD, S], BF, name=f"agin{b}")
                    for b in range(B)]
            agout = [dbounce.tile([H, S], BF, name=f"agout{b}",
                                  addr_space="Shared") for b in range(B)]

            # constants
            idn = cpool.tile([128, 128], F32R, name="idn")
            nc.sync.dma_start(out=idn[:], in_=idn_c[:, :].bitcast(F32R))
            oneskr = cpool.tile([128, 1], F32R, name="oneskr")
            nc.sync.dma_start(out=oneskr[:], in_=onesk_c[:, :].bitcast(F32R))
            onesk = cpool.tile([128, 1], F32, name="onesk")
            nc.sync.dma_start(out=onesk[:], in_=onesk_c[:, :])
            onesmr = cpool.tile([1, 128], F32R, name="onesmr")
            nc.sync.dma_start(out=onesmr[:], in_=onesm_c[:, :].bitcast(F32R))
            onesm = cpool.tile([1, 128], F32, name="onesm")
            nc.sync.dma_start(out=onesm[:], in_=onesm_c[:, :])
            invf = cpool.tile([128, 1], F32, name="invf")
            nc.sync.dma_start(out=invf[:], in_=invf_c[:, :])
            mask_sb = cpool.tile([128, n_blk * 512], F32, name="mask_sb")
            nc.sync.dma_start(
                out=mask_sb[:],
                in_=tiled(maskP[:, :]))
            # broadcast scalars: 0 thq 1 thqn 2 thk 3 thkn 4 thv 5 thvn
            #                    6 cqk 7 gv | 8 tho 9 thon 10 go
            bsc = cpool.tile([128, 12], F32, name="bsc")

            with tc.tile_pool(name="tab", bufs=1) as tab:
                cos_sb = tab.tile([128, T], F32, name="cos_sb")
                ss_sb = tab.tile([128, T], F32, name="ss_sb")
                # RoPE tables: Cody-Waite range reduction + Sin (emitted
                # first so they run during the initial weight DMA).
                with tc.tile_pool(name="rtab", bufs=3) as rtab, \
                     tc.tile_pool(name="prt", bufs=2, space="PSUM") as prt:
                    for tcn in range(NTC):
                        cs = slice(tcn * 512, (tcn + 1) * 512)
                        pchunk = rtab.tile([1, 512], F32, name=f"pos{tcn}",
                                           tag="pos")
                        nc.sync.dma_start(out=pchunk[:], in_=pos[0:1, cs])
                        pf = prt.tile([128, 512], F32, name=f"pf{tcn}",
                                      tag="pf")
                        nc.tensor.matmul(pf[:], onesm[:], pchunk[:],
                                         start=True, stop=True)
                        f_sb = rtab.tile([128, 512], F32, name=f"f{tcn}",
                                         tag="f")
                        nc.scalar.activation(f_sb[:], pf[:], ACTF.Copy,
                                             scale=invf[:])
                        k_sb = rtab.tile([128, 512], F32, name=f"kk{tcn}",
                                         tag="kk")
                        nc.vector.tensor_scalar(k_sb[:], f_sb[:],
                                                1.0 / TWO_PI, C_MAGIC,
                                                ALU.mult, ALU.add)
                        nc.vector.tensor_scalar(k_sb[:], k_sb[:], C_MAGIC,
                                                None, ALU.subtract)
                        y_sb = rtab.tile([128, 512], F32, name=f"y{tcn}",
                                         tag="y")
                        nc.vector.scalar_tensor_tensor(
                            y_sb[:], k_sb[:], -c1, f_sb[:], ALU.mult,
                            ALU.add)
                        nc.vector.scalar_tensor_tensor(
                            y_sb[:], k_sb[:], -c2, y_sb[:], ALU.mult,
                            ALU.add)
                        nc.vector.scalar_tensor_tensor(
                            y_sb[:], k_sb[:], -c3, y_sb[:], ALU.mult,
                            ALU.add)
                        nc.scalar.activation(ss_sb[0:64, cs], y_sb[0:64, :],
                                             ACTF.Sin, scale=-1.0)
                        nc.scalar.activation(ss_sb[64:128, cs],
                                             y_sb[64:128, :], ACTF.Sin)
                        yc = rtab.tile([128, 512], F32, name=f"yc{tcn}",
                                       tag="yc")
                        nc.vector.tensor_scalar(yc[:], y_sb[:],
                                                float(np.pi / 2), None,
                                                ALU.add)
                        m_sb = rtab.tile([128, 512], F32, name=f"mm{tcn}",
                                         tag="mm")
                        nc.vector.tensor_scalar(m_sb[:], yc[:],
                                                float(np.pi), None,
                                                ALU.is_gt)
                        nc.vector.scalar_tensor_tensor(
                            yc[:], m_sb[:], -TWO_PI, yc[:], ALU.mult,
                            ALU.add)
                        nc.scalar.activation(cos_sb[:, cs], yc[:], ACTF.Sin)

                # ---- pools ordered by lifetime for LIFO release ----
                gacc = tc.alloc_tile_pool(name="gacc", bufs=1)
                w3 = tc.alloc_tile_pool(name="w3", bufs=1)
                wo_sb = w3.tile([128, NKT * MSH], BF, name="wo_sb")
                qkvp = tc.alloc_tile_pool(name="qkv", bufs=1)
                qT_sb = [qkvp.tile([128, T], BF, name=f"qT{h}")
                         for h in range(QH)]
                kT_sb = qkvp.tile([128, T], BF, name="kT_sb")
                vT_sb = qkvp.tile([128, T], F32R, name="vT_sb")
                wbig = tc.alloc_tile_pool(name="wbig", bufs=1)
                wq_sb = wbig.tile([128, NKT * 512], BF, name="wq_sb")
                wbig2 = tc.alloc_tile_pool(name="wbig2", bufs=1)
                wk_sb = wbig2.tile([128, NKT * 128], BF, name="wk_sb")
                wv_sb = wbig2.tile([128, NKT * 128], BF, name="wv_sb")

                # ---- gamma prepass: Wq/Wo streamed, Wk/Wv staged ----
                wstage = tc.alloc_tile_pool(name="wstage", bufs=1)
                wk_f = wstage.tile([128, NKT * 128], F32, name="wk_f")
                wv_f = wstage.tile([128, NKT * 128], F32, name="wv_f")

                accq = gacc.tile([128, NKT], F32, name="accq")
                acck = gacc.tile([128, NKT], F32, name="acck")
                accv = gacc.tile([128, NKT], F32, name="accv")
                acco = gacc.tile([128, NKT], F32, name="acco")
                g4 = gacc.tile([128, 4], F32, name="g4")

                with tc.tile_pool(name="wqpre", bufs=2) as wqpre, \
                     tc.tile_pool(name="wopre", bufs=2) as wopre:
                    for j in range(8):
                        sl = wqpre.tile([128, 4 * 512], F32, name=f"wqp{j}",
                                        tag="wqp")
                        nc.sync.dma_start(
                            out=sl[:],
                            in_=tiled(wqT[j * 512:(j + 1) * 512, :]))
                        for i in range(4):
                            nc.vector.tensor_reduce(
                                accq[:, j * 4 + i:j * 4 + i + 1],
                                sl[:, i * 512:(i + 1) * 512], X, ALU.add,
                                apply_absolute_value=True)
                    nc.sync.dma_start(
                        out=wk_f[:],
                        in_=tiled(wkT[:, :]))
                    nc.sync.dma_start(
                        out=wv_f[:],
                        in_=tiled(wvT[:, :]))
                    for i in range(NKT):
                        nc.vector.tensor_reduce(
                            acck[:, i:i + 1],
                            wk_f[:, i * 128:(i + 1) * 128], X, ALU.add,
                            apply_absolute_value=True)
                        nc.vector.tensor_reduce(
                            accv[:, i:i + 1],
                            wv_f[:, i * 128:(i + 1) * 128], X, ALU.add,
                            apply_absolute_value=True)
                    # Wo |.| sums ride along up front too
                    for j in range(8):
                        sl = wopre.tile([128, 4 * 512], F32, name=f"wop{j}",
                                        tag="wop")
                        nc.gpsimd.dma_start(
                            out=sl[:],
                            in_=tiled(woT[j * 512:(j + 1) * 512, :]))
                        for i in range(4):
                            nc.vector.tensor_reduce(
                                acco[:, j * 4 + i:j * 4 + i + 1],
                                sl[:, i * 512:(i + 1) * 512], X, ALU.add,
                                apply_absolute_value=True)
                    nc.vector.tensor_reduce(g4[:, 0:1], accq[:], X, ALU.add)
                    nc.vector.tensor_reduce(g4[:, 1:2], acck[:], X, ALU.add)
                    nc.vector.tensor_reduce(g4[:, 2:3], accv[:], X, ALU.add)
                    nc.vector.tensor_reduce(g4[:, 3:4], acco[:], X, ALU.add)

                def quant_tile(pool, src, dst, thp, thn, tg):
                    scr = pool.tile([128, src.shape[1]], F32,
                                    name=f"qs_{tg}", tag=f"qs{tg[0]}")
                    nc.vector.tensor_scalar(scr[:], src, thn, None,
                                            ALU.is_lt)
                    nc.vector.scalar_tensor_tensor(
                        dst, src, thp, scr[:], ALU.is_gt, ALU.subtract)

                with tc.tile_pool(name="pgam", bufs=1, space="PSUM") \
                        as pgam:
                    # AllReduce #1: q/k/v gamma sums
                    pg_q = pgam.tile([1, 3], F32, name="pg_q", tag="pg")
                    nc.tensor.matmul(pg_q[:], onesk[:], g4[:, 0:3],
                                     start=True, stop=True)
                    gq_sb = gacc.tile([1, 8], F32, name="gq_sb")
                    nc.vector.memset(gq_sb[:], 0.0)
                    nc.scalar.copy(gq_sb[:, 0:3], pg_q[:])
                    nc.sync.dma_start(out=arq_in[:], in_=gq_sb[:])
                    nc.gpsimd.collective_compute(
                        "AllReduce", ALU.add, replica_groups=RG,
                        ins=[arq_in[:].opt()], outs=[arq_out[:].opt()])
                    arq_sb = gacc.tile([1, 8], F32, name="arq_sb")
                    nc.scalar.dma_start(out=arq_sb[:], in_=arq_out[:])

                    # AllReduce #2: Wo gamma sum (issued right behind #1;
                    # result consumed mid-phase-1)
                    pg_o = pgam.tile([1, 1], F32, name="pg_o", tag="pg")
                    nc.tensor.matmul(pg_o[:], onesk[:], g4[:, 3:4],
                                     start=True, stop=True)
                    go_sb = gacc.tile([1, 8], F32, name="go_sb")
                    nc.vector.memset(go_sb[:], 0.0)
                    nc.scalar.copy(go_sb[:, 0:1], pg_o[:])
                    nc.scalar.dma_start(out=aro_in[:], in_=go_sb[:])
                    nc.gpsimd.collective_compute(
                        "AllReduce", ALU.add, replica_groups=RG,
                        ins=[aro_in[:].opt()], outs=[aro_out[:].opt()])

                    # q/k/v gammas -> thresholds / folded scales
                    gam = gacc.tile([1, 8], F32, name="gam")
                    nc.vector.tensor_scalar(gam[:, 0:1], arq_sb[:, 0:1],
                                            1.0 / NQ, 1e-5, ALU.mult,
                                            ALU.add)
                    nc.vector.tensor_scalar(gam[:, 1:3], arq_sb[:, 1:3],
                                            1.0 / NK, 1e-5, ALU.mult,
                                            ALU.add)
                    bcin = gacc.tile([1, 8], F32, name="bcin")
                    nc.vector.tensor_scalar(bcin[:, 0:1], gam[:, 0:1], 0.5,
                                            None, ALU.mult)
                    nc.vector.tensor_scalar(bcin[:, 1:2], gam[:, 0:1], -0.5,
                                            None, ALU.mult)
                    nc.vector.tensor_scalar(bcin[:, 2:3], gam[:, 1:2], 0.5,
                                            None, ALU.mult)
                    nc.vector.tensor_scalar(bcin[:, 3:4], gam[:, 1:2], -0.5,
                                            None, ALU.mult)
                    nc.vector.tensor_scalar(bcin[:, 4:5], gam[:, 2:3], 0.5,
                                            None, ALU.mult)
                    nc.vector.tensor_scalar(bcin[:, 5:6], gam[:, 2:3], -0.5,
                                            None, ALU.mult)
                    nc.vector.tensor_mul(bcin[:, 6:7], gam[:, 0:1],
                                         gam[:, 1:2])
                    nc.vector.tensor_scalar(bcin[:, 6:7], bcin[:, 6:7], ISQ,
                                            None, ALU.mult)
                    nc.vector.tensor_copy(bcin[:, 7:8], gam[:, 2:3])
                    pgb1 = pgam.tile([128, 8], F32, name="pgb1", tag="pg")
                    nc.tensor.matmul(pgb1[:], onesm[:], bcin[:], start=True,
                                     stop=True)
                    nc.scalar.copy(bsc[:, 0:8], pgb1[:])

                    THQ, THQN = bsc[:, 0:1], bsc[:, 1:2]
                    THK, THKN = bsc[:, 2:3], bsc[:, 3:4]
                    THV, THVN = bsc[:, 4:5], bsc[:, 5:6]
                    CQK, GV = bsc[:, 6:7], bsc[:, 7:8]
                    THO, THON, GO = bsc[:, 8:9], bsc[:, 9:10], bsc[:, 10:11]

                    # ---- quant q/k/v -> exact ternary bf16 (Wq re-read
                    # in slabs that prefetch during the AllReduce wait)
                    with tc.tile_pool(name="wqst", bufs=2) as wqst, \
                         tc.tile_pool(name="qscr", bufs=4) as qscr:
                        for j in range(8):
                            sl = wqst.tile([128, 4 * 512], F32,
                                           name=f"wq2_{j}", tag="wq2")
                            nc.sync.dma_start(
                                out=sl[:],
                                in_=tiled(
                                    wqT[j * 512:(j + 1) * 512, :]))
                            for i in range(4):
                                k = j * 4 + i
                                quant_tile(qscr,
                                           sl[:, i * 512:(i + 1) * 512],
                                           wq_sb[:, k * 512:(k + 1) * 512],
                                           THQ, THQN, f"q{k}")
                                quant_tile(qscr,
                                           wk_f[:, k * 128:(k + 1) * 128],
                                           wk_sb[:, k * 128:(k + 1) * 128],
                                           THK, THKN, f"k{k}")
                                quant_tile(qscr,
                                           wv_f[:, k * 128:(k + 1) * 128],
                                           wv_sb[:, k * 128:(k + 1) * 128],
                                           THV, THVN, f"v{k}")
                wstage.release()

                # ---- phase 1: QKV projections + RoPE + Wo quant ----
                with tc.tile_pool(name="xin", bufs=3) as xin, \
                     tc.tile_pool(name="rope", bufs=2) as rope, \
                     tc.tile_pool(name="wop2", bufs=2) as wop2, \
                     tc.tile_pool(name="qsc2", bufs=2) as qsc2, \
                     tc.tile_pool(name="p1", bufs=8, space="PSUM") as p1:
                    for tcn in range(NTC):
                        cs = slice(tcn * 512, (tcn + 1) * 512)
                        xsl = []
                        for j in range(4):
                            sl = xin.tile([128, 8 * 512], BF,
                                          name=f"x{tcn}_{j}", tag="xt")
                            nc.sync.dma_start(out=sl[:],
                                              in_=slab(xT, j, 512, cs))
                            xsl.append(sl)
                        pq = [p1.tile([128, 512], F32, name=f"pq{tcn}_{h}",
                                      tag="p1") for h in range(QH)]
                        pk = p1.tile([128, 512], F32, name=f"pk{tcn}",
                                     tag="p1")
                        pv = p1.tile([128, 512], F32, name=f"pv{tcn}",
                                     tag="p1")
                        for kt in range(NKT):
                            xt_ = xsl[kt // 8][:, (kt % 8) * 512:
                                               (kt % 8 + 1) * 512]
                            st, sp = (kt == 0), (kt == NKT - 1)
                            for h in range(QH):
                                nc.tensor.matmul(
                                    pq[h][:],
                                    wq_sb[:, kt * 512 + h * 128:
                                          kt * 512 + (h + 1) * 128],
                                    xt_, start=st, stop=sp,
                                    skip_group_check=True)
                            nc.tensor.matmul(
                                pk[:], wk_sb[:, kt * 128:(kt + 1) * 128],
                                xt_, start=st, stop=sp,
                                skip_group_check=True)
                            nc.tensor.matmul(
                                pv[:], wv_sb[:, kt * 128:(kt + 1) * 128],
                                xt_, start=st, stop=sp,
                                skip_group_check=True)

                        def rope_apply(psrc, dst_ap, tg):
                            m1 = rope.tile([128, 512], F32, name=f"m1{tg}",
                                           tag="m1")
                            nc.vector.tensor_mul(m1[:], psrc[:],
                                                 cos_sb[:, cs])
                            m2 = rope.tile([128, 512], F32, name=f"m2{tg}",
                                           tag="m2")
                            nc.vector.tensor_mul(m2[0:64, :],
                                                 psrc[64:128, :],
                                                 ss_sb[0:64, cs])
                            nc.vector.tensor_mul(m2[64:128, :],
                                                 psrc[0:64, :],
                                                 ss_sb[64:128, cs])
                            nc.vector.tensor_add(dst_ap, m1[:], m2[:])

                        for h in range(QH):
                            rope_apply(pq[h], qT_sb[h][:, cs], f"_{tcn}_{h}")
                        rope_apply(pk, kT_sb[:, cs], f"k_{tcn}")
                        nc.scalar.activation(vT_sb[:, cs], pv[:], ACTF.Copy,
                                             scale=GV)

                        if tcn == 1:
                            # Wo gamma -> thresholds (AR2 done long ago;
                            # emitted here so nothing upstream stalls)
                            aro_sb = gacc.tile([1, 8], F32, name="aro_sb")
                            nc.scalar.dma_start(out=aro_sb[:],
                                                in_=aro_out[:])
                            gamo = gacc.tile([1, 3], F32, name="gamo")
                            nc.vector.tensor_scalar(gamo[:, 2:3],
                                                    aro_sb[:, 0:1],
                                                    1.0 / NO, 1e-5,
                                                    ALU.mult, ALU.add)
                            nc.vector.tensor_scalar(gamo[:, 0:1],
                                                    gamo[:, 2:3], 0.5,
                                                    None, ALU.mult)
                            nc.vector.tensor_scalar(gamo[:, 1:2],
                                                    gamo[:, 2:3], -0.5,
                                                    None, ALU.mult)
                            pgb2 = p1.tile([128, 512], F32, name="pgb2",
                                           tag="p1")
                            nc.tensor.matmul(pgb2[:, 0:3], onesm[:],
                                             gamo[:], start=True, stop=True)
                            nc.scalar.copy(bsc[:, 8:11], pgb2[:, 0:3])

                        if tcn >= 2:
                            # Wo quant rides along in the vector slack
                            jj = tcn - 2
                            for j2 in range(4):
                                j = jj * 4 + j2
                                wt = wop2.tile([128, 4 * 512], F32,
                                               name=f"wo2_{j}", tag="wo2")
                                nc.scalar.dma_start(
                                    out=wt[:],
                                    in_=tiled(
                                        woT[j * 512:(j + 1) * 512, :]))
                                for i in range(4):
                                    k = j * 4 + i
                                    quant_tile(
                                        qsc2,
                                        wt[:, i * 512:(i + 1) * 512],
                                        wo_sb[:, k * MSH:(k + 1) * MSH],
                                        THO, THON, f"o{k}")

                wbig2.release()
                wbig.release()

                # ---- phase 2: attention ----
                with tc.tile_pool(name="vnatp", bufs=2) as vnatp, \
                     tc.tile_pool(name="epool", bufs=8) as epool, \
                     tc.tile_pool(name="aop", bufs=4) as aop, \
                     tc.tile_pool(name="zpool", bufs=2) as zpool, \
                     tc.tile_pool(name="ps_s", bufs=4,
                                  space="PSUM") as ps_s, \
                     tc.tile_pool(name="ps_o", bufs=2,
                                  space="PSUM") as ps_o, \
                     tc.tile_pool(name="ps_x", bufs=1,
                                  space="PSUM") as ps_x:
                    for b in range(B):
                        boff = b * S
                        vnat = vnatp.tile([128, S], F32R, name=f"vnat{b}",
                                          tag="vnat")
                        for kt in range(SKT):
                            ptr = ps_x.tile([128, 128], F32R,
                                            name=f"ptr{b}_{kt}", tag="misc")
                            nc.tensor.transpose(
                                ptr[:],
                                vT_sb[:, boff + kt * 128:
                                      boff + (kt + 1) * 128], idn[:])
                            nc.scalar.copy(
                                vnat[:, kt * 128:(kt + 1) * 128],
                                ptr[:].bitcast(F32))
                        for qc in range(SQC):
                            kts = [kt for kt in range(SKT)
                                   if status[b, kt, qc] != 1]
                            assert kts, "fully-masked softmax row"
                            for h in range(QH):
                                qsl = qT_sb[h][:, boff + qc * 512:
                                               boff + (qc + 1) * 512]
                                # all score matmuls first: the PE streams
                                # them while mask+exp trail behind
                                es = []
                                for kt in kts:
                                    ps_ = ps_s.tile([128, 512], F32,
                                                    name=f"s{b}{h}{qc}{kt}",
                                                    tag="ps")
                                    nc.tensor.matmul(
                                        ps_[:],
                                        kT_sb[:, boff + kt * 128:
                                              boff + (kt + 1) * 128],
                                        qsl, start=True, stop=True,
                                        skip_group_check=True)
                                    if status[b, kt, qc] == 2:
                                        mi = blk_idx[(b, kt, qc)]
                                        nc.vector.tensor_add(
                                            ps_[:], ps_[:],
                                            mask_sb[:, mi * 512:
                                                    (mi + 1) * 512])
                                    e = epool.tile([128, 512], F32R,
                                                   name=f"e{b}{h}{qc}{kt}",
                                                   tag="e")
                                    nc.scalar.activation(e[:], ps_[:],
                                                         ACTF.Exp,
                                                         scale=CQK)
                                    es.append(e)
                                pz = ps_x.tile([1, 512], F32,
                                               name=f"pz{b}{h}{qc}",
                                               tag="pz")
                                po = ps_o.tile([128, 512], F32,
                                               name=f"po{b}{h}{qc}",
                                               tag="po")
                                for i, kt in enumerate(kts):
                                    fst = (i == 0)
                                    lst = (i == len(kts) - 1)
                                    nc.tensor.matmul(
                                        pz[:], oneskr[:], es[i][:],
                                        start=fst, stop=lst,
                                        skip_group_check=True)
                                    nc.tensor.matmul(
                                        po[:],
                                        vnat[:, kt * 128:(kt + 1) * 128],
                                        es[i][:], start=fst, stop=lst,
                                        skip_group_check=True)
                                zr = zpool.tile([1, 512], F32R,
                                                name=f"zr{b}{h}{qc}",
                                                tag="zr")
                                nc.vector.reciprocal(zr[:], pz[:])
                                pzb = ps_x.tile([128, 512], F32,
                                                name=f"pzb{b}{h}{qc}",
                                                tag="misc")
                                nc.tensor.matmul(pzb[:], onesmr[:], zr[:],
                                                 start=True, stop=True,
                                                 skip_group_check=True)
                                zb = zpool.tile([128, 512], F32,
                                                name=f"zb{b}{h}{qc}",
                                                tag="zb")
                                nc.scalar.copy(zb[:], pzb[:])
                                ao = aop.tile([128, 512], BF,
                                              name=f"ao{b}{h}{qc}",
                                              tag="ao")
                                nc.vector.tensor_mul(ao[:], po[:], zb[:])
                                nc.sync.dma_start(
                                    out=agin[b][h * 128:(h + 1) * 128,
                                                qc * 512:(qc + 1) * 512],
                                    in_=ao[:])
                        nc.gpsimd.collective_compute(
                            "AllGather", ALU.bypass, replica_groups=RG,
                            ins=[agin[b][:].opt()],
                            outs=[agout[b][:].opt()])

                # ---- phase 3: o_proj, split per batch for AG overlap ----
                with tc.tile_pool(name="a3", bufs=2) as a3, \
                     tc.tile_pool(name="o3", bufs=2) as o3, \
                     tc.tile_pool(name="p3", bufs=4, space="PSUM") as p3:
                    for ch in range(NTC):
                        b, q2 = ch // 2, ch % 2
                        at = a3.tile([128, NKT * 512], BF, name=f"at{ch}",
                                     tag="at")
                        nc.sync.dma_start(
                            out=at[:],
                            in_=tiled(
                                agout[b][:, q2 * 512:(q2 + 1) * 512]))
                        for tt in range(4):
                            pout = p3.tile([128, 512], F32,
                                           name=f"po3_{ch}{tt}", tag="pout")
                            for kt in range(NKT):
                                nc.tensor.matmul(
                                    pout[:],
                                    at[:, kt * 512 + tt * 128:
                                       kt * 512 + (tt + 1) * 128],
                                    wo_sb[:, kt * MSH:(kt + 1) * MSH],
                                    start=(kt == 0), stop=(kt == NKT - 1),
                                    skip_group_check=True)
                            osb = o3.tile([128, 512], F32,
                                          name=f"osb{ch}{tt}", tag="osb")
                            nc.scalar.activation(osb[:], pout[:], ACTF.Copy,
                                                 scale=GO)
                            nc.sync.dma_start(
                                out=outN[ch * 512 + tt * 128:
                                         ch * 512 + (tt + 1) * 128, :],
                                in_=osb[:])
                qkvp.release()
                w3.release()
                gacc.release()

    nc.compile()
    return nc


def kernel(hidden_states, Wq, Wk, Wv, Wo, attention_mask, position_ids):
    from concourse.bass_utils import run_bass_kernel_spmd
    from concourse.bass_interp import get_hw_module

    hs = np.ascontiguousarray(np.asarray(hidden_states, dtype=np.float32))
    Wq = np.asarray(Wq, dtype=np.float32)
    Wk = np.asarray(Wk, dtype=np.float32)
    Wv = np.asarray(Wv, dtype=np.float32)
    Wo = np.asarray(Wo, dtype=np.float32)
    mask = np.asarray(attention_mask, dtype=np.float32)
    posf = np.ascontiguousarray(
        np.asarray(position_ids).reshape(1, T).astype(np.float32))

    status, blk_idx, packed = _classify_mask(mask)
    n_blk = packed.shape[0] // 128
    assert n_blk <= 16, "too many distinct mask blocks"

    key = (status.tobytes(), tuple(sorted(blk_idx.items())), n_blk)
    if key not in _cache:
        nc = _build(status, blk_idx, n_blk)
        nc.m = get_hw_module(nc.m)
        _cache[key] = nc
    nc = _cache[key]

    xT = np.ascontiguousarray(hs.reshape(T, H).T.astype(BF16NP))
    in_maps = []
    for c in range(NCORES):
        in_maps.append({
            "xT": xT,
            "wqT": np.ascontiguousarray(
                Wq[c * QH * HD:(c + 1) * QH * HD, :].T),
            "wkT": np.ascontiguousarray(Wk[c * HD:(c + 1) * HD, :].T),
            "wvT": np.ascontiguousarray(Wv[c * HD:(c + 1) * HD, :].T),
            "woT": np.ascontiguousarray(Wo[c * MSH:(c + 1) * MSH, :].T),
            "maskP": packed,
            "pos": posf,
        })
    res = run_bass_kernel_spmd(nc, in_maps, core_ids=list(range(NCORES)),
                               trace=bool(os.environ.get("BITNET_TRACE")))
    global last_exec_time_ns
    last_exec_time_ns = res.exec_time_ns
    out = np.concatenate(
        [res.results[c]["outN"] for c in range(NCORES)], axis=1)  # (T, MSH*8)
    return np.ascontiguousarray(out).reshape(B, S, H).astype(np.float32)


# revision 14
# speedup vs baseline: 1.3401x; 1.0491x over previous
"""BitNet attention block on 8 TRN2 NeuronCores (tensor-parallel over heads).

Self-contained: kernel(**inputs) takes full inputs, shards internally,
runs one SPMD Bass program on cores 0-7, reassembles the full output.

Sharding: core c owns Q heads [4c,4c+4), KV head c, o_proj output dims
[512c, 512c+512). Attention is fully local per core. Cross-core comms:
two tiny AllReduces for the BitNet absmean gammas and four bf16
AllGathers (one per (batch, 512-token chunk)) of the attention output.

Key optimizations vs the fp32r baseline:
- Exact ternary {-1,0,+1} weights stored bf16; gamma scales folded into
  activation `scale=` APs (exp carries gq*gk/sqrt(HD), the V copy gv,
  the o_proj copy go). x is bf16 (host-converted). Quant decisions
  reproduce round-half-even+clip via wq = (w > g/2) - (w < -g/2).
- Everything the PE streams in the hot loops is bf16 (fp32r moving
  operands measure ~2x slower on HW than the cost model claims).
- The causal mask is added on the PE itself (an accumulating
  identity @ mask matmul into the score PSUM group) so the
  score->exp chain never crosses through the Vector engine.
- Softmax normalization: ones-broadcast matmul of the PSUM z-row,
  then a full-width [128,512] reciprocal (a [1,512] reciprocal is
  single-lane and costs 3.3us).
- Gamma AllReduce results are read back with a partition-broadcast
  DMA so thresholds are pure Vector work (no PE/Scalar hops on the
  critical path). AR2-dependent Wo work is emitted mid-phase-1.
- DMA descriptor issue costs ~0.6us, so all tile loads ride
  multi-tile slab DMAs built with AP rearrange+transpose.
- Phase 3 for batch b overlaps the other batch's AllGathers; the
  per-qc AllGather split lets agout land earlier.
"""
import os
import sys
sys.path.insert(0, "/opt/trn_rl_repo")
import numpy as np
import ml_dtypes

B, S, H = 2, 1024, 4096
NH, NKV, HD = 32, 8, 128
NCORES = 8
T = B * S
QH = NH // NCORES          # 4 q-heads per core
MSH = H // NCORES          # 512 o_proj out-dims per core
THETA = 10000.0
C_MAGIC = 12582912.0       # 1.5 * 2**23
TWO_PI = 6.283185307179586
NKT = H // 128             # 32 contraction tiles
NTC = T // 512             # 4 token chunks
SKT = S // 128             # 8 score k-tiles per batch
SQC = S // 512             # 2 q-chunks per batch
BF16NP = ml_dtypes.bfloat16

_cache = {}
last_exec_time_ns = None


def _classify_mask(mask):
    """Per (b, kt, qc) [128k x 512q] block: 0 no-op, 1 fully masked
    (skipped), 2 needs a mask add (index into deduped distinct blocks)."""
    status = np.empty((B, SKT, SQC), dtype=np.int8)
    blk_idx = {}
    distinct = []
    seen = {}
    for b in range(B):
        mb = np.asarray(mask[b, 0], dtype=np.float32)
        for kt in range(SKT):
            for qc in range(SQC):
                blk = mb[qc * 512:(qc + 1) * 512, kt * 128:(kt + 1) * 128]
                if not blk.any():
                    status[b, kt, qc] = 0
                elif (blk <= -1e4).all():
                    status[b, kt, qc] = 1
                else:
                    status[b, kt, qc] = 2
                    kb = blk.tobytes()
                    if kb not in seen:
                        seen[kb] = len(distinct)
                        distinct.append(np.ascontiguousarray(blk.T))
                    blk_idx[(b, kt, qc)] = seen[kb]
    if distinct:
        packed = np.concatenate(distinct, axis=0)
    else:
        packed = np.zeros((128, 512), dtype=np.float32)
    return status, blk_idx, np.ascontiguousarray(packed.astype(BF16NP))


def _cody_consts():
    c1 = float(np.float32(6.28125))
    r = np.float64(TWO_PI) - c1
    c2 = float(np.float32(r - np.remainder(r, 2.0 ** -24)))
    c3 = float(np.float32(np.float64(TWO_PI) - c1 - float(c2)))
    return c1, c2, c3


def _build(status, blk_idx, n_blk):
    from concourse import bacc, tile, mybir

    F32 = mybir.dt.float32
    F32R = mybir.dt.float32r
    BF = mybir.dt.bfloat16
    ACTF = mybir.ActivationFunctionType
    ALU = mybir.AluOpType
    X = mybir.AxisListType.X
    RG = [list(range(NCORES))]
    c1, c2, c3 = _cody_consts()

    nc = bacc.Bacc("TRN2", target_bir_lowering=False, debug=False,
                   num_devices=NCORES)

    xT = nc.dram_tensor("xT", [H, T], BF, kind="ExternalInput")
    wqT = nc.dram_tensor("wqT", [H, QH * HD], F32, kind="ExternalInput")
    wkT = nc.dram_tensor("wkT", [H, HD], F32, kind="ExternalInput")
    wvT = nc.dram_tensor("wvT", [H, HD], F32, kind="ExternalInput")
    woT = nc.dram_tensor("woT", [H, MSH], F32, kind="ExternalInput")
    maskP = nc.dram_tensor("maskP", [n_blk * 128, 512], BF,
                           kind="ExternalInput")
    pos = nc.dram_tensor("pos", [1, T], F32, kind="ExternalInput")
    outN = nc.dram_tensor("outN", [T, MSH], F32, kind="ExternalOutput")

    idnb_c = nc.inline_tensor(np.eye(128, dtype=BF16NP), name="idnb_c")
    onesm_c = nc.inline_tensor(np.ones((1, 128), np.float32), name="onesm_c")
    onesk_c = nc.inline_tensor(np.ones((128, 1), np.float32), name="onesk_c")
    invf_np = (1.0 / THETA ** (np.arange(0, HD, 2, dtype=np.float32) / HD))
    invf_np = np.concatenate([invf_np, invf_np]).reshape(HD, 1)
    invf_c = nc.inline_tensor(invf_np.astype(np.float32), name="invf_c")

    NQ = float(NH * HD * H)
    NK = float(NKV * HD * H)
    NO = float(H * NH * HD)
    ISQ = float(1.0 / np.sqrt(HD))

    def tiled(src):
        """[(i 128), c] DRAM slice -> [128, i, c] AP (partition-major)."""
        return src.rearrange("(i p) c -> i p c", p=128).transpose([1, 0, 2])

    with tile.TileContext(nc) as tc, \
         nc.allow_low_precision(reason="bf16 ternary kernel"):
        with tc.tile_pool(name="cpool", bufs=1) as cpool, \
             tc.tile_pool(name="dbounce", bufs=1, space="DRAM") as dbounce:
            # DRAM bounce tiles for the collectives
            arq_in = dbounce.tile([1, 8], F32, name="arq_in")
            arq_out = dbounce.tile([1, 8], F32, name="arq_out",
                                   addr_space="Shared")
            aro_in = dbounce.tile([1, 8], F32, name="aro_in")
            aro_out = dbounce.tile([1, 8], F32, name="aro_out",
                                   addr_space="Shared")
            agin = [[dbounce.tile([QH * HD, 512], BF, name=f"agi{b}{qc}")
                     for qc in range(SQC)] for b in range(B)]
            agout = [[dbounce.tile([H, 512], BF, name=f"ago{b}{qc}",
                                   addr_space="Shared")
                      for qc in range(SQC)] for b in range(B)]

            # constants
            idnb = cpool.tile([128, 128], BF, name="idnb")
            nc.sync.dma_start(out=idnb[:], in_=idnb_c[:, :])
            oneskb = cpool.tile([128, 1], BF, name="oneskb")
            nc.vector.memset(oneskb[:], 1.0)
            onesk = cpool.tile([128, 1], F32, name="onesk")
            nc.sync.dma_start(out=onesk[:], in_=onesk_c[:, :])
            onesmr = cpool.tile([1, 128], F32R, name="onesmr")
            nc.sync.dma_start(out=onesmr[:], in_=onesm_c[:, :].bitcast(F32R))
            invf = cpool.tile([128, 1], F32, name="invf")
            nc.sync.dma_start(out=invf[:], in_=invf_c[:, :])
            mask_sb = cpool.tile([128, n_blk * 512], BF, name="mask_sb")
            nc.sync.dma_start(out=mask_sb[:], in_=tiled(maskP[:, :]))
            # broadcast scalars: 0 thq 1 thqn 2 thk 3 thkn 4 thv 5 thvn
            #                    6 cqk 7 gv | 8 tho 9 thon 10 go
            bsc = cpool.tile([128, 12], F32, name="bsc")
            gw = cpool.tile([128, 4], F32, name="gw")
            arq128 = cpool.tile([128, 8], F32, name="arq128")
            aro128 = cpool.tile([128, 8], F32, name="aro128")

            with tc.tile_pool(name="tab", bufs=1) as tab:
                cos_sb = tab.tile([128, T], F32, name="cos_sb")
                ss_sb = tab.tile([128, T], F32, name="ss_sb")
                # RoPE tables: Cody-Waite range reduction + Sin (emitted
                # first so they run during the initial weight DMA).
                with tc.tile_pool(name="rtab", bufs=3) as rtab:
                    for tcn in range(NTC):
                        cs = slice(tcn * 512, (tcn + 1) * 512)
                        pf = rtab.tile([128, 512], F32, name=f"pf{tcn}",
                                       tag="pf")
                        nc.scalar.dma_start(
                            out=pf[:],
                            in_=pos[0:1, cs].partition_broadcast(128))
                        f_sb = rtab.tile([128, 512], F32, name=f"f{tcn}",
                                         tag="f")
                        nc.scalar.activation(f_sb[:], pf[:], ACTF.Copy,
                                             scale=invf[:])
                        k_sb = rtab.tile([128, 512], F32, name=f"kk{tcn}",
                                         tag="kk")
                        nc.vector.tensor_scalar(k_sb[:], f_sb[:],
                                                1.0 / TWO_PI, C_MAGIC,
                                                ALU.mult, ALU.add)
                        nc.vector.tensor_scalar(k_sb[:], k_sb[:], C_MAGIC,
                                                None, ALU.subtract)
                        y_sb = rtab.tile([128, 512], F32, name=f"y{tcn}",
                                         tag="y")
                        nc.vector.scalar_tensor_tensor(
                            y_sb[:], k_sb[:], -c1, f_sb[:], ALU.mult,
                            ALU.add)
                        nc.vector.scalar_tensor_tensor(
                            y_sb[:], k_sb[:], -c2, y_sb[:], ALU.mult,
                            ALU.add)
                        nc.vector.scalar_tensor_tensor(
                            y_sb[:], k_sb[:], -c3, y_sb[:], ALU.mult,
                            ALU.add)
                        nc.scalar.activation(ss_sb[0:64, cs], y_sb[0:64, :],
                                             ACTF.Sin, scale=-1.0)
                        nc.scalar.activation(ss_sb[64:128, cs],
                                             y_sb[64:128, :], ACTF.Sin)
                        yc = rtab.tile([128, 512], F32, name=f"yc{tcn}",
                                       tag="yc")
                        nc.vector.tensor_scalar(yc[:], y_sb[:],
                                                float(np.pi / 2), None,
                                                ALU.add)
                        m_sb = rtab.tile([128, 512], F32, name=f"mm{tcn}",
                                         tag="mm")
                        nc.vector.tensor_scalar(m_sb[:], yc[:],
                                                float(np.pi), None,
                                                ALU.is_gt)
                        nc.vector.scalar_tensor_tensor(
                            yc[:], m_sb[:], -TWO_PI, yc[:], ALU.mult,
                            ALU.add)
                        nc.scalar.activation(cos_sb[:, cs], yc[:], ACTF.Sin)

                # ---- pools ordered by lifetime for LIFO release ----
                gacc = tc.alloc_tile_pool(name="gacc", bufs=1)
                w3 = tc.alloc_tile_pool(name="w3", bufs=1)
                wo_sb = w3.tile([128, NKT * MSH], BF, name="wo_sb")
                qkvp = tc.alloc_tile_pool(name="qkv", bufs=1)
                qT_sb = [qkvp.tile([128, T], BF, name=f"qT{h}")
                         for h in range(QH)]
                kT_sb = qkvp.tile([128, T], BF, name="kT_sb")
                vT_sb = qkvp.tile([128, T], BF, name="vT_sb")
                wbig = tc.alloc_tile_pool(name="wbig", bufs=1)
                wq_sb = wbig.tile([128, NKT * 512], BF, name="wq_sb")
                wbig2 = tc.alloc_tile_pool(name="wbig2", bufs=1)
                wk_sb = wbig2.tile([128, NKT * 128], BF, name="wk_sb")
                wv_sb = wbig2.tile([128, NKT * 128], BF, name="wv_sb")

                # ---- gamma prepass: Wq/Wo streamed, Wk/Wv staged ----
                wstage = tc.alloc_tile_pool(name="wstage", bufs=1)
                wk_f = wstage.tile([128, NKT * 128], F32, name="wk_f")
                wv_f = wstage.tile([128, NKT * 128], F32, name="wv_f")

                accq = gacc.tile([128, NKT], F32, name="accq")
                acck = gacc.tile([128, NKT], F32, name="acck")
                accv = gacc.tile([128, NKT], F32, name="accv")
                acco = gacc.tile([128, NKT], F32, name="acco")
                g4 = gacc.tile([128, 4], F32, name="g4")

                with tc.tile_pool(name="wqpre", bufs=3) as wqpre, \
                     tc.tile_pool(name="wopre", bufs=4) as wopre:
                    for j in range(16):
                        sl = wqpre.tile([128, 2 * 512], F32, name=f"wqp{j}",
                                        tag="wqp")
                        nc.sync.dma_start(
                            out=sl[:],
                            in_=tiled(wqT[j * 256:(j + 1) * 256, :]))
                        for i in range(2):
                            nc.vector.tensor_reduce(
                                accq[:, j * 2 + i:j * 2 + i + 1],
                                sl[:, i * 512:(i + 1) * 512], X, ALU.add,
                                apply_absolute_value=True)
                    nc.sync.dma_start(out=wk_f[:], in_=tiled(wkT[:, :]))
                    nc.sync.dma_start(out=wv_f[:], in_=tiled(wvT[:, :]))
                    for i in range(NKT):
                        nc.vector.tensor_reduce(
                            acck[:, i:i + 1],
                            wk_f[:, i * 128:(i + 1) * 128], X, ALU.add,
                            apply_absolute_value=True)
                        nc.vector.tensor_reduce(
                            accv[:, i:i + 1],
                            wv_f[:, i * 128:(i + 1) * 128], X, ALU.add,
                            apply_absolute_value=True)
                    # Wo |.| sums ride along up front too
                    for j in range(8):
                        sl = wopre.tile([128, 4 * 512], F32, name=f"wop{j}",
                                        tag="wop")
                        nc.gpsimd.dma_start(
                            out=sl[:],
                            in_=tiled(woT[j * 512:(j + 1) * 512, :]))
                        for i in range(4):
                            nc.vector.tensor_reduce(
                                acco[:, j * 4 + i:j * 4 + i + 1],
                                sl[:, i * 512:(i + 1) * 512], X, ALU.add,
                                apply_absolute_value=True)
                    nc.vector.tensor_reduce(g4[:, 0:1], accq[:], X, ALU.add)
                    nc.vector.tensor_reduce(g4[:, 1:2], acck[:], X, ALU.add)
                    nc.vector.tensor_reduce(g4[:, 2:3], accv[:], X, ALU.add)
                    nc.vector.tensor_reduce(g4[:, 3:4], acco[:], X, ALU.add)

                with tc.tile_pool(name="pgam", bufs=1, space="PSUM") \
                        as pgam:
                    # AllReduce #1: q/k/v gamma sums
                    pg_q = pgam.tile([1, 3], F32, name="pg_q", tag="pg")
                    nc.tensor.matmul(pg_q[:], onesk[:], g4[:, 0:3],
                                     start=True, stop=True)
                    gq_sb = gacc.tile([1, 8], F32, name="gq_sb")
                    nc.vector.memset(gq_sb[:], 0.0)
                    nc.scalar.copy(gq_sb[:, 0:3], pg_q[:])
                    nc.sync.dma_start(out=arq_in[:], in_=gq_sb[:])
                    nc.gpsimd.collective_compute(
                        "AllReduce", ALU.add, replica_groups=RG,
                        ins=[arq_in[:].opt()], outs=[arq_out[:].opt()])

                    # AllReduce #2: Wo gamma sum (issued right behind #1;
                    # result consumed mid-phase-1)
                    pg_o = pgam.tile([1, 1], F32, name="pg_o", tag="pg")
                    nc.tensor.matmul(pg_o[:], onesk[:], g4[:, 3:4],
                                     start=True, stop=True)
                    go_sb = gacc.tile([1, 8], F32, name="go_sb")
                    nc.vector.memset(go_sb[:], 0.0)
                    nc.scalar.copy(go_sb[:, 0:1], pg_o[:])
                    nc.scalar.dma_start(out=aro_in[:], in_=go_sb[:])
                    nc.gpsimd.collective_compute(
                        "AllReduce", ALU.add, replica_groups=RG,
                        ins=[aro_in[:].opt()], outs=[aro_out[:].opt()])

                # partition-broadcast readback: thresholds become pure
                # Vector work, no PE/Scalar hops before quant can start
                nc.scalar.dma_start(
                    out=arq128[:],
                    in_=arq_out[:, :].partition_broadcast(128))
                nc.vector.tensor_scalar(gw[:, 0:1], arq128[:, 0:1],
                                        1.0 / NQ, 1e-5, ALU.mult, ALU.add)
                nc.vector.tensor_scalar(gw[:, 1:3], arq128[:, 1:3],
                                        1.0 / NK, 1e-5, ALU.mult, ALU.add)
                nc.vector.tensor_scalar(bsc[:, 0:1], gw[:, 0:1], 0.5,
                                        None, ALU.mult)
                nc.vector.tensor_scalar(bsc[:, 1:2], gw[:, 0:1], -0.5,
                                        None, ALU.mult)
                nc.vector.tensor_scalar(bsc[:, 2:3], gw[:, 1:2], 0.5,
                                        None, ALU.mult)
                nc.vector.tensor_scalar(bsc[:, 3:4], gw[:, 1:2], -0.5,
                                        None, ALU.mult)
                nc.vector.tensor_scalar(bsc[:, 4:5], gw[:, 2:3], 0.5,
                                        None, ALU.mult)
                nc.vector.tensor_scalar(bsc[:, 5:6], gw[:, 2:3], -0.5,
                                        None, ALU.mult)
                nc.vector.tensor_mul(bsc[:, 6:7], gw[:, 0:1], gw[:, 1:2])
                nc.vector.tensor_scalar(bsc[:, 6:7], bsc[:, 6:7], ISQ,
                                        None, ALU.mult)
                nc.vector.tensor_copy(bsc[:, 7:8], gw[:, 2:3])

                THQ, THQN = bsc[:, 0:1], bsc[:, 1:2]
                THK, THKN = bsc[:, 2:3], bsc[:, 3:4]
                THV, THVN = bsc[:, 4:5], bsc[:, 5:6]
                CQK, GV = bsc[:, 6:7], bsc[:, 7:8]
                THO, THON, GO = bsc[:, 8:9], bsc[:, 9:10], bsc[:, 10:11]

                def quant_tile(pool, src, dst, thp, thn, tg):
                    scr = pool.tile([128, src.shape[1]], F32,
                                    name=f"qs_{tg}", tag=f"qs{tg[0]}")
                    nc.vector.tensor_scalar(scr[:], src, thn, None,
                                            ALU.is_lt)
                    nc.vector.scalar_tensor_tensor(
                        dst, src, thp, scr[:], ALU.is_gt, ALU.subtract)

                # ---- quant q/k/v -> exact ternary bf16 (Wq re-read in
                # slabs that prefetch during the AllReduce wait)
                with tc.tile_pool(name="wqst", bufs=3) as wqst, \
                     tc.tile_pool(name="qscr", bufs=4) as qscr:
                    for j in range(16):
                        sl = wqst.tile([128, 2 * 512], F32,
                                       name=f"wq2_{j}", tag="wq2")
                        nc.sync.dma_start(
                            out=sl[:],
                            in_=tiled(wqT[j * 256:(j + 1) * 256, :]))
                        for i in range(2):
                            k = j * 2 + i
                            quant_tile(qscr, sl[:, i * 512:(i + 1) * 512],
                                       wq_sb[:, k * 512:(k + 1) * 512],
                                       THQ, THQN, f"q{k}")
                            quant_tile(qscr,
                                       wk_f[:, k * 128:(k + 1) * 128],
                                       wk_sb[:, k * 128:(k + 1) * 128],
                                       THK, THKN, f"k{k}")
                            quant_tile(qscr,
                                       wv_f[:, k * 128:(k + 1) * 128],
                                       wv_sb[:, k * 128:(k + 1) * 128],
                                       THV, THVN, f"v{k}")
                wstage.release()

                # ---- phase 1: QKV projections + RoPE + Wo quant ----
                with tc.tile_pool(name="xin", bufs=4) as xin, \
                     tc.tile_pool(name="rope", bufs=2) as rope, \
                     tc.tile_pool(name="wop2", bufs=2) as wop2, \
                     tc.tile_pool(name="qsc2", bufs=2) as qsc2, \
                     tc.tile_pool(name="p1", bufs=8, space="PSUM") as p1:
                    for tcn in range(NTC):
                        cs = slice(tcn * 512, (tcn + 1) * 512)
                        xsl = []
                        for j in range(4):
                            sl = xin.tile([128, 8 * 512], BF,
                                          name=f"x{tcn}_{j}", tag="xt")
                            nc.sync.dma_start(
                                out=sl[:],
                                in_=tiled(xT[j * 1024:(j + 1) * 1024, cs]))
                            xsl.append(sl)
                        pq = [p1.tile([128, 512], F32, name=f"pq{tcn}_{h}",
                                      tag="p1") for h in range(QH)]
                        pk = p1.tile([128, 512], F32, name=f"pk{tcn}",
                                     tag="p1")
                        pv = p1.tile([128, 512], F32, name=f"pv{tcn}",
                                     tag="p1")
                        for kt in range(NKT):
                            xt_ = xsl[kt // 8][:, (kt % 8) * 512:
                                               (kt % 8 + 1) * 512]
                            st, sp = (kt == 0), (kt == NKT - 1)
                            for h in range(QH):
                                nc.tensor.matmul(
                                    pq[h][:],
                                    wq_sb[:, kt * 512 + h * 128:
                                          kt * 512 + (h + 1) * 128],
                                    xt_, start=st, stop=sp,
                                    skip_group_check=True)
                            nc.tensor.matmul(
                                pk[:], wk_sb[:, kt * 128:(kt + 1) * 128],
                                xt_, start=st, stop=sp,
                                skip_group_check=True)
                            nc.tensor.matmul(
                                pv[:], wv_sb[:, kt * 128:(kt + 1) * 128],
                                xt_, start=st, stop=sp,
                                skip_group_check=True)

                        def rope_apply(psrc, dst_ap, tg):
                            m1 = rope.tile([128, 512], F32, name=f"m1{tg}",
                                           tag="m1")
                            nc.vector.tensor_mul(m1[:], psrc[:],
                                                 cos_sb[:, cs])
                            m2 = rope.tile([128, 512], F32, name=f"m2{tg}",
                                           tag="m2")
                            nc.vector.tensor_mul(m2[0:64, :],
                                                 psrc[64:128, :],
                                                 ss_sb[0:64, cs])
                            nc.vector.tensor_mul(m2[64:128, :],
                                                 psrc[0:64, :],
                                                 ss_sb[64:128, cs])
                            nc.vector.tensor_add(dst_ap, m1[:], m2[:])

                        for h in range(QH):
                            rope_apply(pq[h], qT_sb[h][:, cs], f"_{tcn}_{h}")
                        rope_apply(pk, kT_sb[:, cs], f"k_{tcn}")
                        nc.scalar.activation(vT_sb[:, cs], pv[:], ACTF.Copy,
                                             scale=GV)

                        if tcn == 1:
                            # Wo gamma -> thresholds (AR2 done long ago)
                            nc.scalar.dma_start(
                                out=aro128[:],
                                in_=aro_out[:, :].partition_broadcast(128))
                            nc.vector.tensor_scalar(gw[:, 3:4],
                                                    aro128[:, 0:1],
                                                    1.0 / NO, 1e-5,
                                                    ALU.mult, ALU.add)
                            nc.vector.tensor_scalar(bsc[:, 8:9],
                                                    gw[:, 3:4], 0.5,
                                                    None, ALU.mult)
                            nc.vector.tensor_scalar(bsc[:, 9:10],
                                                    gw[:, 3:4], -0.5,
                                                    None, ALU.mult)
                            nc.vector.tensor_copy(bsc[:, 10:11],
                                                  gw[:, 3:4])

                        if tcn >= 2:
                            # Wo quant rides along in the vector slack
                            jj = tcn - 2
                            for j2 in range(4):
                                j = jj * 4 + j2
                                wt = wop2.tile([128, 4 * 512], F32,
                                               name=f"wo2_{j}", tag="wo2")
                                nc.scalar.dma_start(
                                    out=wt[:],
                                    in_=tiled(
                                        woT[j * 512:(j + 1) * 512, :]))
                                for i in range(4):
                                    k = j * 4 + i
                                    quant_tile(
                                        qsc2,
                                        wt[:, i * 512:(i + 1) * 512],
                                        wo_sb[:, k * MSH:(k + 1) * MSH],
                                        THO, THON, f"o{k}")

                wbig2.release()
                wbig.release()

                # ---- phase 2: attention (all-bf16 PE path) ----
                with tc.tile_pool(name="vnatp", bufs=2) as vnatp, \
                     tc.tile_pool(name="epool", bufs=8) as epool, \
                     tc.tile_pool(name="aop", bufs=4) as aop, \
                     tc.tile_pool(name="zpool", bufs=2) as zpool, \
                     tc.tile_pool(name="ps_s", bufs=4,
                                  space="PSUM") as ps_s, \
                     tc.tile_pool(name="ps_o", bufs=2,
                                  space="PSUM") as ps_o, \
                     tc.tile_pool(name="ps_x", bufs=1,
                                  space="PSUM") as ps_x:
                    for b in range(B):
                        boff = b * S
                        vnat = vnatp.tile([128, S], BF, name=f"vnat{b}",
                                          tag="vnat")
                        for kt in range(SKT):
                            ptr = ps_o.tile([128, 512], BF,
                                            name=f"ptr{b}_{kt}", tag="po")
                            nc.tensor.transpose(
                                ptr[:, 0:128],
                                vT_sb[:, boff + kt * 128:
                                      boff + (kt + 1) * 128], idnb[:])
                            nc.vector.tensor_copy(
                                vnat[:, kt * 128:(kt + 1) * 128],
                                ptr[:, 0:128])
                        for qc in range(SQC):
                            kts = [kt for kt in range(SKT)
                                   if status[b, kt, qc] != 1]
                            assert kts, "fully-masked softmax row"
                            for h in range(QH):
                                qsl = qT_sb[h][:, boff + qc * 512:
                                               boff + (qc + 1) * 512]
                                # scores stream on the PE; the causal mask
                                # is added by an accumulating idn @ mask
                                # matmul (no Vector hop in the chain)
                                es = []
                                for kt in kts:
                                    masked = status[b, kt, qc] == 2
                                    ps_ = ps_s.tile([128, 512], F32,
                                                    name=f"s{b}{h}{qc}{kt}",
                                                    tag="ps")
                                    nc.tensor.matmul(
                                        ps_[:],
                                        kT_sb[:, boff + kt * 128:
                                              boff + (kt + 1) * 128],
                                        qsl, start=True, stop=not masked,
                                        skip_group_check=True)
                                    if masked:
                                        mi = blk_idx[(b, kt, qc)]
                                        nc.tensor.matmul(
                                            ps_[:], idnb[:],
                                            mask_sb[:, mi * 512:
                                                    (mi + 1) * 512],
                                            start=False, stop=True,
                                            skip_group_check=True)
                                    e = epool.tile([128, 512], BF,
                                                   name=f"e{b}{h}{qc}{kt}",
                                                   tag="e")
                                    nc.scalar.activation(e[:], ps_[:],
                                                         ACTF.Exp,
                                                         scale=CQK)
                                    es.append(e)
                                pz = ps_x.tile([1, 512], F32,
                                               name=f"pz{b}{h}{qc}",
                                               tag="pz")
                                po = ps_o.tile([128, 512], F32,
                                               name=f"po{b}{h}{qc}",
                                               tag="po")
                                for i, kt in enumerate(kts):
                                    fst = (i == 0)
                                    lst = (i == len(kts) - 1)
                                    nc.tensor.matmul(
                                        pz[:], oneskb[:], es[i][:],
                                        start=fst, stop=lst,
                                        skip_group_check=True)
                                    nc.tensor.matmul(
                                        po[:],
                                        vnat[:, kt * 128:(kt + 1) * 128],
                                        es[i][:], start=fst, stop=lst,
                                        skip_group_check=True)
                                # z -> SBUF, ones-broadcast, full-width
                                # reciprocal (a [1,512] recip is 1-lane)
                                zsb = zpool.tile([1, 512], F32R,
                                                 name=f"zs{b}{h}{qc}",
                                                 tag="zs")
                                nc.scalar.copy(zsb[:], pz[:])
                                pzb = ps_x.tile([128, 512], F32,
                                                name=f"pzb{b}{h}{qc}",
                                                tag="pzb")
                                nc.tensor.matmul(pzb[:], onesmr[:], zsb[:],
                                                 start=True, stop=True,
                                                 skip_group_check=True)
                                zb = zpool.tile([128, 512], F32,
                                                name=f"zb{b}{h}{qc}",
                                                tag="zb")
                                nc.vector.reciprocal(zb[:], pzb[:])
                                ao = aop.tile([128, 512], BF,
                                              name=f"ao{b}{h}{qc}",
                                              tag="ao")
                                nc.vector.tensor_mul(ao[:], po[:], zb[:])
                                nc.sync.dma_start(
                                    out=agin[b][qc][h * 128:(h + 1) * 128,
                                                    :],
                                    in_=ao[:])
                            nc.gpsimd.collective_compute(
                                "AllGather", ALU.bypass, replica_groups=RG,
                                ins=[agin[b][qc][:].opt()],
                                outs=[agout[b][qc][:].opt()])

                # ---- phase 3: o_proj, per (batch, qchunk) for overlap ----
                with tc.tile_pool(name="a3", bufs=2) as a3, \
                     tc.tile_pool(name="o3", bufs=2) as o3, \
                     tc.tile_pool(name="p3", bufs=4, space="PSUM") as p3:
                    for ch in range(NTC):
                        b, q2 = ch // 2, ch % 2
                        at = a3.tile([128, NKT * 512], BF, name=f"at{ch}",
                                     tag="at")
                        nc.sync.dma_start(out=at[:],
                                          in_=tiled(agout[b][q2][:, :]))
                        for tt in range(4):
                            pout = p3.tile([128, 512], F32,
                                           name=f"po3_{ch}{tt}", tag="pout")
                            for kt in range(NKT):
                                nc.tensor.matmul(
                                    pout[:],
                                    at[:, kt * 512 + tt * 128:
                                       kt * 512 + (tt + 1) * 128],
                                    wo_sb[:, kt * MSH:(kt + 1) * MSH],
                                    start=(kt == 0), stop=(kt == NKT - 1),
                                    skip_group_check=True)
                            osb = o3.tile([128, 512], F32,
                                          name=f"osb{ch}{tt}", tag="osb")
                            nc.scalar.activation(osb[:], pout[:], ACTF.Copy,
                                                 scale=GO)
                            nc.sync.dma_start(
                                out=outN[ch * 512 + tt * 128:
                                         ch * 512 + (tt + 1) * 128, :],
                                in_=osb[:])
                qkvp.release()
                w3.release()
                gacc.release()

    nc.compile()
    return nc


def kernel(hidden_states, Wq, Wk, Wv, Wo, attention_mask, position_ids):
    from concourse.bass_utils import run_bass_kernel_spmd
    from concourse.bass_interp import get_hw_module

    hs = np.ascontiguousarray(np.asarray(hidden_states, dtype=np.float32))
    Wq = np.asarray(Wq, dtype=np.float32)
    Wk = np.asarray(Wk, dtype=np.float32)
    Wv = np.asarray(Wv, dtype=np.float32)
    Wo = np.asarray(Wo, dtype=np.float32)
    mask = np.asarray(attention_mask, dtype=np.float32)
    posf = np.ascontiguousarray(
        np.asarray(position_ids).reshape(1, T).astype(np.float32))

    status, blk_idx, packed = _classify_mask(mask)
    n_blk = packed.shape[0] // 128
    assert n_blk <= 16, "too many distinct mask blocks"

    key = (status.tobytes(), tuple(sorted(blk_idx.items())), n_blk)
    if key not in _cache:
        nc = _build(status, blk_idx, n_blk)
        nc.m = get_hw_module(nc.m)
        _cache[key] = nc
    nc = _cache[key]

    xT = np.ascontiguousarray(hs.reshape(T, H).T.astype(BF16NP))
    in_maps = []
    for c in range(NCORES):
        in_maps.append({
            "xT": xT,
            "wqT": np.ascontiguousarray(
                Wq[c * QH * HD:(c + 1) * QH * HD, :].T),
            "wkT": np.ascontiguousarray(Wk[c * HD:(c + 1) * HD, :].T),
            "wvT": np.ascontiguousarray(Wv[c * HD:(c + 1) * HD, :].T),
            "woT": np.ascontiguousarray(Wo[c * MSH:(c + 1) * MSH, :].T),
            "maskP": packed,
            "pos": posf,
        })
    res = run_bass_kernel_spmd(nc, in_maps, core_ids=list(range(NCORES)),
                               trace=bool(os.environ.get("BITNET_TRACE")))
    global last_exec_time_ns
    last_exec_time_ns = res.exec_time_ns
    out = np.concatenate(
        [res.results[c]["outN"] for c in range(NCORES)], axis=1)  # (T, MSH*8)
    return np.ascontiguousarray(out).reshape(B, S, H).astype(np.float32)
